# revision 1
# baseline (speedup 1.0000x reference)
"""Cross-attention Bass kernel for 8 trn2 NeuronCores.

Sharding: core d handles batch b = d//4 and query rows [(d%4)*1024, (d%4+1)*1024)
of that batch, computing all 8 heads (no collectives needed). The context is
compacted on the host using the mask (masked rows dropped, zero-padded to a
fixed M_PAD), which exactly preserves softmax semantics while halving the
score-matrix work.

Device dataflow (feature-major layouts, f32r matmuls):
  x^T, ctx^T via PE transposes -> Q^T = Wq^T x^T (scaled by 1/sqrt(D)),
  K^T = Wk^T ctx^T, V natural = ctx Wv with a per-head "ones" column carrying
  the valid mask. Scores computed transposed S^T[k, q] = K^T_h-chunks.T @ Q^T,
  exp on ScalarE straight out of multi-bank PSUM, P^T @ [V | valid] accumulates
  attention output AND softmax denominators in one matmul. Normalization
  broadcasts 1/l across partitions via a DRAM round-trip. Output projection
  consumes O^T directly and emits the natural [q, e] layout.

Engine placement: during the projection prologue ScalarE is otherwise idle, so
all PSUM->SBUF drains run there, keeping VectorE free and PSUM slots cycling
fast; during attention ScalarE does the exps and VectorE handles normalize.
"""
import numpy as np

B, N, M = 2, 4096, 4096
QUERY_DIM, CONTEXT_DIM = 512, 768
H, D = 8, 64
INNER = H * D  # 512
NCORES = 8
N_DEV = (B * N) // NCORES  # 1024 query rows per core
M_PAD_MIN = 2304  # 18 k-tiles; P(Binomial(4096,.5) > 2304) ~ 1e-15

_compiled = {}


def _build(m_pad):
    from concourse import bacc
    import concourse.bass as bass
    import concourse.mybir as mybir
    import concourse.tile as tile
    from concourse.masks import make_identity

    F32 = mybir.dt.float32
    F32R = mybir.dt.float32r
    AF = mybir.ActivationFunctionType

    KT_TILES = m_pad // 128  # 18
    KF = [(s, min(512, m_pad - s)) for s in range(0, m_pad, 512)]
    SC_G = 3  # k-tiles per exp instruction (3 PSUM banks)
    GROUPS = [(g, min(SC_G, KT_TILES - g)) for g in range(0, KT_TILES, SC_G)]
    QB = 512  # q-block (free dim of score matmuls)
    NQB = N_DEV // QB  # 2
    SCALE = float(D) ** -0.5

    nc = bacc.Bacc()
    xs_d = nc.declare_dram_parameter("xs", [N_DEV, QUERY_DIM], F32, isOutput=False)
    ctx_d = nc.declare_dram_parameter("ctx", [m_pad, CONTEXT_DIM], F32, isOutput=False)
    val_d = nc.declare_dram_parameter("valid", [m_pad], F32, isOutput=False)
    wq_d = nc.declare_dram_parameter("Wq", [QUERY_DIM, INNER], F32, isOutput=False)
    wk_d = nc.declare_dram_parameter("Wk", [CONTEXT_DIM, INNER], F32, isOutput=False)
    wv_d = nc.declare_dram_parameter("Wv", [CONTEXT_DIM, INNER], F32, isOutput=False)
    wo_d = nc.declare_dram_parameter("Wo", [INNER, QUERY_DIM], F32, isOutput=False)
    bo_d = nc.declare_dram_parameter("bo", [QUERY_DIM], F32, isOutput=False)
    out_d = nc.declare_dram_parameter("out", [N_DEV, QUERY_DIM], F32, isOutput=True)

    rec_scratch = nc.dram_tensor("rec_scratch", [NQB * H, 512], F32)

    CQ = QUERY_DIM // 128  # 4
    CC = CONTEXT_DIM // 128  # 6
    CI = INNER // 128  # 4

    with tile.TileContext(nc) as tc:
        with (
            tc.tile_pool(name="big", bufs=1) as big,
            tc.tile_pool(name="wts", bufs=1) as wts,
            tc.tile_pool(name="ps_sc", bufs=2, space="PSUM") as ps_sc,
            tc.tile_pool(name="ps_pv", bufs=2, space="PSUM") as ps_pv,
        ):
            qT = big.tile([128, CI, N_DEV], F32R, tag="qT", name="qT")
            kTb = [
                big.tile([128, CI, bw], F32R, tag=f"kT{i}", name=f"kT{i}")
                for i, (base, bw) in enumerate(KF)
            ]
            v2t = [
                big.tile([128, H * 65], F32R, tag=f"v2_{t}", name=f"v2_{t}")
                for t in range(KT_TILES)
            ]
            oTq = [
                big.tile([128, CI, QB], F32R, tag=f"oT{qb}", name=f"oT{qb}")
                for qb in range(NQB)
            ]
            wo = wts.tile([128, CI, QUERY_DIM], F32R, tag="wo", name="wo")
            bo_bc = wts.tile([128, QUERY_DIM], F32, tag="bo", name="bo")
            nc.sync.dma_start(
                out=bo_bc[:],
                in_=bass.AP(tensor=bo_d, offset=0, ap=[[0, 128], [1, QUERY_DIM]]),
            )
            valid = wts.tile([128, KT_TILES], F32, tag="valid", name="valid")
            nc.sync.dma_start(
                out=valid[:], in_=val_d[:].rearrange("(t p) -> p t", p=128)
            )

            # ======== prologue: projections (scoped pools) ========
            with (
                tc.tile_pool(name="pwts", bufs=1) as pwts,
                tc.tile_pool(name="ld", bufs=4) as ld,
                tc.tile_pool(name="ctxt", bufs=2) as ctxt,
            ):
                identf = pwts.tile([128, 128], F32, tag="identf", name="identf")
                make_identity(nc, identf[:])
                ident = pwts.tile([128, 128], F32R, tag="ident", name="ident")
                nc.vector.tensor_copy(ident[:], identf[:])
                wq = pwts.tile([128, CQ, INNER], F32R, tag="wq", name="wq")
                wk = pwts.tile([128, CC, INNER], F32R, tag="wk", name="wk")
                wv = pwts.tile([128, CC, INNER], F32R, tag="wv", name="wv")
                xT = pwts.tile([128, CQ, N_DEV], F32R, tag="xT", name="xT")

                # x^T: 4 transposes share one PSUM slot, one strided ACT drain
                for nt in range(N_DEV // 128):
                    x_tile = ld.tile(
                        [128, QUERY_DIM], F32R, tag="x_tile", name="x_tile"
                    )
                    nc.gpsimd.dma_start(
                        out=x_tile[:], in_=xs_d[nt * 128 : (nt + 1) * 128, :]
                    )
                    pst = ps_sc.tile([128, 3 * QB], F32R, tag="sc", name="pst")
                    for c in range(CQ):
                        nc.tensor.transpose(
                            pst[:, c * 128 : (c + 1) * 128],
                            x_tile[:, c * 128 : (c + 1) * 128],
                            ident[:],
                        )
                    nc.vector.tensor_copy(
                        xT[:, :, nt * 128 : (nt + 1) * 128],
                        pst[:, 0 : CQ * 128].rearrange("p (c n) -> p c n", n=128),
                    )

                # Q^T (softmax scale folded into the ACT drain)
                nc.gpsimd.dma_start(
                    out=wq[:], in_=wq_d[:].rearrange("(o p) f -> p o f", p=128)
                )
                for dc in range(CI):
                    for qf in range(N_DEV // 512):
                        psq = ps_pv.tile([128, 512], F32, tag="pv", name="psq")
                        for c in range(CQ):
                            nc.tensor.matmul(
                                psq[:],
                                wq[:, c, dc * 128 : (dc + 1) * 128],
                                xT[:, c, qf * 512 : (qf + 1) * 512],
                                start=(c == 0),
                                stop=(c == CQ - 1),
                            )
                        nc.scalar.activation(
                            qT[:, dc, qf * 512 : (qf + 1) * 512], psq[:], AF.Copy,
                            scale=SCALE,
                        )

                # ctx^T, K^T, V'' per 512-wide context block
                nc.gpsimd.dma_start(
                    out=wk[:], in_=wk_d[:].rearrange("(o p) f -> p o f", p=128)
                )
                nc.gpsimd.dma_start(
                    out=wv[:], in_=wv_d[:].rearrange("(o p) f -> p o f", p=128)
                )
                for bi, (base, bw) in enumerate(KF):
                    nkt = bw // 128
                    ctxT = ctxt.tile([128, CC, 512], F32R, tag="ctxT", name="ctxT")
                    for kt in range(nkt):
                        c_tile = ld.tile(
                            [128, CONTEXT_DIM], F32R, tag="c_tile", name="c_tile"
                        )
                        nc.gpsimd.dma_start(
                            out=c_tile[:],
                            in_=ctx_d[base + kt * 128 : base + (kt + 1) * 128, :],
                        )
                        pst = ps_sc.tile([128, 3 * QB], F32R, tag="sc", name="pst2")
                        for c in range(CC):
                            nc.tensor.transpose(
                                pst[:, c * 128 : (c + 1) * 128],
                                c_tile[:, c * 128 : (c + 1) * 128],
                                ident[:],
                            )
                        nc.vector.tensor_copy(
                            ctxT[:, :, kt * 128 : (kt + 1) * 128],
                            pst[:, 0 : CC * 128].rearrange("p (c n) -> p c n", n=128),
                        )
                    for dc in range(CI):
                        psk = ps_pv.tile([128, 512], F32, tag="pv", name="psk")
                        for c in range(CC):
                            nc.tensor.matmul(
                                psk[:, :bw],
                                wk[:, c, dc * 128 : (dc + 1) * 128],
                                ctxT[:, c, :bw],
                                start=(c == 0),
                                stop=(c == CC - 1),
                            )
                        nc.scalar.activation(kTb[bi][:, dc, :], psk[:, :bw], AF.Copy)
                    for kt in range(nkt):
                        t = base // 128 + kt
                        psv = ps_pv.tile([128, 512], F32, tag="pv", name="psv")
                        for c in range(CC):
                            nc.tensor.matmul(
                                psv[:],
                                ctxT[:, c, kt * 128 : (kt + 1) * 128],
                                wv[:, c, :],
                                start=(c == 0),
                                stop=(c == CC - 1),
                            )
                        v2h = v2t[t][:].rearrange("p (h c) -> p h c", c=65)
                        nc.scalar.activation(
                            v2h[:, :, 0:64],
                            psv[:].rearrange("p (h d) -> p h d", d=64),
                            AF.Copy,
                        )
                        nc.vector.tensor_copy(
                            v2h[:, :, 64:65],
                            valid[:, t : t + 1].to_broadcast([128, H, 1]),
                        )

            # ======== attention ========
            nc.gpsimd.dma_start(
                out=wo[:], in_=wo_d[:].rearrange("(o p) f -> p o f", p=128)
            )
            with (
                tc.tile_pool(name="pt", bufs=3) as ptp,
                tc.tile_pool(name="sm", bufs=3) as sm,
                tc.tile_pool(name="outp", bufs=3) as outp,
            ):
                def out_proj(qb):
                    for qtl in range(QB // 128):
                        qt = qb * (QB // 128) + qtl
                        pso = ps_pv.tile([128, 512], F32, tag="pv", name="pso")
                        for c in range(CI):
                            nc.tensor.matmul(
                                pso[:],
                                oTq[qb][:, c, qtl * 128 : (qtl + 1) * 128],
                                wo[:, c, :],
                                start=(c == 0),
                                stop=(c == CI - 1),
                            )
                        ot = outp.tile([128, QUERY_DIM], F32, tag="ot", name="ot")
                        nc.vector.tensor_add(ot[:], pso[:], bo_bc[:])
                        nc.sync.dma_start(
                            out=out_d[qt * 128 : (qt + 1) * 128, :], in_=ot[:]
                        )

                for qb in range(NQB):
                    q0 = qb * QB
                    for hp in range(H // 2):
                        hA, hB = 2 * hp, 2 * hp + 1
                        pvA = ps_pv.tile([128, 512], F32, tag="pv", name="pvA")
                        pvB = ps_pv.tile([128, 512], F32, tag="pv", name="pvB")
                        for g0, gn in GROUPS:
                            scA = ps_sc.tile([128, 3 * QB], F32, tag="sc", name="scA")
                            scB = ps_sc.tile([128, 3 * QB], F32, tag="sc", name="scB")
                            for j in range(gn):
                                kt = g0 + j
                                bi, co = kt // 4, (kt % 4) * 128
                                nc.tensor.matmul(
                                    scA[:, j * QB : (j + 1) * QB],
                                    kTb[bi][0:64, hp, co : co + 128],
                                    qT[0:64, hp, q0 : q0 + QB],
                                    start=True,
                                    stop=True,
                                )
                                nc.tensor.matmul(
                                    scB[:, j * QB : (j + 1) * QB],
                                    kTb[bi][64:128, hp, co : co + 128],
                                    qT[64:128, hp, q0 : q0 + QB],
                                    start=True,
                                    stop=True,
                                )
                            ptA = ptp.tile([128, 3 * QB], F32R, tag="pt", name="ptA")
                            ptB = ptp.tile([128, 3 * QB], F32R, tag="pt", name="ptB")
                            nc.scalar.activation(
                                ptA[:, : gn * QB], scA[:, : gn * QB], AF.Exp
                            )
                            nc.scalar.activation(
                                ptB[:, : gn * QB], scB[:, : gn * QB], AF.Exp
                            )
                            for j in range(gn):
                                kt = g0 + j
                                nc.tensor.matmul(
                                    pvA[:65, :],
                                    v2t[kt][:, hA * 65 : hA * 65 + 65],
                                    ptA[:, j * QB : (j + 1) * QB],
                                    start=(kt == 0),
                                    stop=(kt == KT_TILES - 1),
                                )
                                nc.tensor.matmul(
                                    pvB[:65, :],
                                    v2t[kt][:, hB * 65 : hB * 65 + 65],
                                    ptB[:, j * QB : (j + 1) * QB],
                                    start=(kt == 0),
                                    stop=(kt == KT_TILES - 1),
                                )
                        # normalize: oT_h = pv[0:64] * broadcast(1/pv[64])
                        for h, pv in ((hA, pvA), (hB, pvB)):
                            pvs = sm.tile([65, 512], F32, tag="pvs", name="pvs")
                            nc.vector.tensor_copy(pvs[:], pv[:65, :])
                            nc.vector.reciprocal(pvs[64:65, :], pvs[64:65, :])
                            sl = qb * H + h
                            nc.sync.dma_start(
                                out=rec_scratch[sl : sl + 1, :], in_=pvs[64:65, :]
                            )
                            bcs = sm.tile([64, 512], F32, tag="bcs", name="bcs")
                            nc.sync.dma_start(
                                out=bcs[:],
                                in_=bass.AP(
                                    tensor=rec_scratch,
                                    offset=sl * 512,
                                    ap=[[0, 64], [1, 512]],
                                ),
                            )
                            r0 = (h % 2) * 64
                            nc.vector.tensor_mul(
                                oTq[qb][r0 : r0 + 64, h // 2, :],
                                pvs[0:64, :],
                                bcs[:],
                            )
                        if qb == 1 and hp == 1:
                            out_proj(0)
                for qb in range(NQB):
                    if qb == 0:
                        continue
                    out_proj(qb)

    nc.compile()
    return nc


def kernel(x, context_tensor, mask, Wq, Wk, Wv, Wo, bo):
    from concourse.bass_utils import run_bass_kernel_spmd

    x = np.ascontiguousarray(np.asarray(x, dtype=np.float32))
    context_tensor = np.ascontiguousarray(np.asarray(context_tensor, dtype=np.float32))
    mask = np.asarray(mask)
    Wq = np.ascontiguousarray(np.asarray(Wq, dtype=np.float32))
    Wk = np.ascontiguousarray(np.asarray(Wk, dtype=np.float32))
    Wv = np.ascontiguousarray(np.asarray(Wv, dtype=np.float32))
    Wo = np.ascontiguousarray(np.asarray(Wo, dtype=np.float32))
    bo = np.ascontiguousarray(np.asarray(bo, dtype=np.float32))

    # host-side context compaction using the mask
    meffs = [int(mask[b].sum()) for b in range(B)]
    m_pad = max(M_PAD_MIN, ((max(meffs) + 127) // 128) * 128)
    ctx_c = np.zeros((B, m_pad, CONTEXT_DIM), dtype=np.float32)
    val = np.zeros((B, m_pad), dtype=np.float32)
    for b in range(B):
        idx = np.flatnonzero(mask[b])
        ctx_c[b, : len(idx)] = context_tensor[b, idx]
        val[b, : len(idx)] = 1.0

    if m_pad not in _compiled:
        _compiled[m_pad] = _build(m_pad)
    nc = _compiled[m_pad]

    rows_per_core = N // (NCORES // B)  # 1024
    in_maps = []
    for d in range(NCORES):
        b = d // (NCORES // B)
        r0 = (d % (NCORES // B)) * rows_per_core
        in_maps.append(
            {
                "xs": x[b, r0 : r0 + rows_per_core],
                "ctx": ctx_c[b],
                "valid": val[b],
                "Wq": Wq,
                "Wk": Wk,
                "Wv": Wv,
                "Wo": Wo,
                "bo": bo,
            }
        )

    res = run_bass_kernel_spmd(nc, in_maps, list(range(NCORES)))
    out = np.empty((B, N, QUERY_DIM), dtype=np.float32)
    for d in range(NCORES):
        b = d // (NCORES // B)
        r0 = (d % (NCORES // B)) * rows_per_core
        out[b, r0 : r0 + rows_per_core] = res.results[d]["out"]
    return out



# revision 20
# speedup vs baseline: 1.0607x; 1.0607x over previous
"""Cross-attention Bass kernel for 8 trn2 NeuronCores.

Sharding: core d handles batch b = d//4 and query rows [(d%4)*1024, (d%4+1)*1024)
of that batch, computing all 8 heads (no collectives). The context is compacted
on the host using the mask (masked rows dropped, zero-padded to a multiple of
128), which preserves softmax semantics exactly.

Design (v3):
- All data bf16 except: PSUM (f32), scores path (fp8e4 DoubleRow).
- x^T / ctx^T produced by XBAR DMA-transpose straight out of DRAM (inputs are
  host-cast to bf16); weight tiles use the matching (o p) f -> p o f layout.
- Scores: Q^T/K^T drained to fp8 (x16) and partition-folded via small
  SBUF->SBUF DMAs into [32, (h, 2, m)] so one DoubleRow matmul per (head,
  k-tile) contracts all 64 head dims at 0.5 cycles/row.
- P = exp(S) on ACT straight out of 3-bank PSUM score groups into bf16; the
  softmax scale and fp8 prescale fold into the exp. ACT runs ONLY exps.
- PV uses P^T chunks as stationary and V (+ones column for the denominator)
  as the 65-wide moving operand. U accumulates per (qtile,head) in one PSUM
  bank (4 slots, first-write start marks the zero region).
- Two passes over k-groups (A: first 8 k-tiles, B: rest) bound U-psum
  residency, so pass-A rounds start after two context blocks; blocks 2+ are
  produced inside pass-A rounds to keep PE busy under the ACT exp stream.
- Normalization is a per-partition broadcast multiply on DVE (q is partition
  dim after PV); O is DMA-transposed for the output projection.
- Every tile is written-once-read-later at whole-tile granularity (per-block
  / per-half tiles) to avoid false WAR/RAW serialization in Tile's tracker.
"""
import numpy as np
import ml_dtypes

B, N, M = 2, 4096, 4096
QUERY_DIM, CONTEXT_DIM = 512, 768
H, D = 8, 64
INNER = H * D  # 512
NCORES = 8
N_DEV = (B * N) // NCORES  # 1024 query rows per core
M_PAD_MIN = 1792

BF16 = ml_dtypes.bfloat16
SCALE = float(D) ** -0.5
FP8_PRE = 16.0  # fp8 prescale on Q and K; exp scale divides by 16*16

_compiled = {}
_DBG = False


def _build(m_pad):
    from concourse import bacc
    import concourse.bass as bass
    import concourse.mybir as mybir
    import concourse.tile as tile

    F32 = mybir.dt.float32
    BF = mybir.dt.bfloat16
    F8 = mybir.dt.float8e4
    AF = mybir.ActivationFunctionType
    PM = mybir.MatmulPerfMode

    KT = m_pad // 128  # k-tiles (17 for 2176)
    # context blocks of up to 4 k-tiles
    BLOCKS = [(4 * g, min(4, KT - 4 * g)) for g in range((KT + 3) // 4)]
    NB = len(BLOCKS)
    BA = 2  # pass A covers blocks [0, BA), pass B the rest
    CQ = QUERY_DIM // 128  # 4
    CC = CONTEXT_DIM // 128  # 6
    CI = INNER // 128  # 4
    EXP_SCALE = SCALE / (FP8_PRE * FP8_PRE)

    # exp groups of up to 3 k-tiles within each pass (psum: 3 banks x 2 bufs)
    def make_groups(kts):
        out = []
        i = 0
        while i < len(kts):
            out.append(kts[i : i + 3])
            i += 3
        return out

    A_KT = [kt for g0, gn in BLOCKS[:BA] for kt in range(g0, g0 + gn)]
    B_KT = [kt for g0, gn in BLOCKS[BA:] for kt in range(g0, g0 + gn)]
    GR_A = make_groups(A_KT)  # e.g. [[0,1,2],[3,4,5],[6,7]]
    GR_B = make_groups(B_KT)  # e.g. [[8,9,10],[11,12,13],[14,15,16]]

    nc = bacc.Bacc()
    xs_d = nc.declare_dram_parameter("xs", [N_DEV, QUERY_DIM], BF, isOutput=False)
    ctx_d = nc.declare_dram_parameter("ctx", [m_pad, CONTEXT_DIM], BF, isOutput=False)
    val_d = nc.declare_dram_parameter("valid", [m_pad], BF, isOutput=False)
    wqk_d = nc.declare_dram_parameter(
        "Wqk", [QUERY_DIM + CONTEXT_DIM, INNER], BF, isOutput=False
    )
    wvo_d = nc.declare_dram_parameter(
        "Wvo", [CONTEXT_DIM + INNER, INNER], BF, isOutput=False
    )
    bo_d = nc.declare_dram_parameter("bo", [QUERY_DIM], F32, isOutput=False)
    out_d = nc.declare_dram_parameter("out", [N_DEV, QUERY_DIM], F32, isOutput=True)

    with tile.TileContext(nc) as tc:
        with (
            tc.tile_pool(name="big", bufs=1) as big,
            tc.tile_pool(name="pt", bufs=2) as ptp,
            tc.tile_pool(name="fin", bufs=2) as finp,
            tc.tile_pool(name="otp", bufs=2) as otp,
            tc.tile_pool(name="outp", bufs=2) as outp,
            tc.tile_pool(name="ps_sc", bufs=2, space="PSUM") as ps_sc,
            tc.tile_pool(name="ps_u", bufs=1, space="PSUM") as ps_u,
            tc.tile_pool(name="ps_kv", bufs=1, space="PSUM") as ps_kv,
        ):
            # ---- static SBUF tiles (each written once, at one spot) ----
            wqk = big.tile([128, CQ + CC, INNER], BF, tag="wqk", name="wqk")
            wvo = big.tile([128, CC + CI, INNER], BF, tag="wvo", name="wvo")
            wq = wqk[:, 0:CQ, :]
            wk = wqk[:, CQ : CQ + CC, :]
            wv = wvo[:, 0:CC, :]
            wo = wvo[:, CC : CC + CI, :]
            bo_bc = big.tile([128, QUERY_DIM], F32, tag="bo", name="bo")
            valid = big.tile([128, KT], BF, tag="valid", name="valid")
            xTh = [
                big.tile([128, CQ, 512], BF, tag=f"xT{i}", name=f"xT{i}")
                for i in range(2)
            ]
            MA = min(1024, m_pad)  # pass-A context columns
            MB = m_pad - MA
            ctxT0 = big.tile([128, CC, 512], BF, tag="cT0", name="cT0")
            ctxT1 = big.tile([128, CC, 512], BF, tag="cT1", name="cT1")
            ctxTB = big.tile([128, CC, MB], BF, tag="cTB", name="cTB")
            qT8h = [
                big.tile([128, CI, 512], F8, tag=f"qT8_{i}", name=f"qT8_{i}")
                for i in range(2)
            ]
            qT8fh = [
                big.tile([32, H, 2, 512], F8, tag=f"qT8f_{i}", name=f"qT8f_{i}")
                for i in range(2)
            ]
            kT8b = [
                big.tile([128, CI, 512], F8, tag=f"kT8b{i}", name=f"kT8b{i}")
                for i in range(2)
            ]
            kT8B = big.tile([128, CI, MB], F8, tag="kT8B", name="kT8B")
            kT8fb = [
                big.tile([32, H, 2, 512], F8, tag=f"kfb{i}", name=f"kfb{i}")
                for i in range(2)
            ]
            kT8fB = big.tile([32, H, 2, MB], F8, tag="kfB", name="kfB")
            v2t = [
                big.tile([128, H, 65], BF, tag=f"v2_{t}", name=f"v2_{t}")
                for t in range(KT)
            ]
            ua = [
                big.tile([128, 4, 65], F32, tag=f"ua{r}", name=f"ua{r}")
                for r in range(16)
            ]
            onat = [
                big.tile([128, 4, INNER], BF, tag=f"on{qb}", name=f"on{qb}")
                for qb in range(2)
            ]

            # ---- loads + input transposes (DMA device is serialized: Q path
            # first, then ctx blocks in consumption order) ----
            # Few, large DMAs: chains between unlike DMAs then cost little.
            nc.sync.dma_start_transpose(xTh[0][:], xs_d[0:512, :])
            nc.sync.dma_start_transpose(xTh[1][:], xs_d[512:1024, :])
            nc.sync.dma_start_transpose(ctxT0[:], ctx_d[0:512, :])
            nc.scalar.dma_start(
                out=wqk[:], in_=wqk_d[:].rearrange("(o p) f -> p o f", p=128)
            )
            nc.scalar.dma_start(
                out=valid[:], in_=val_d[:].rearrange("(t p) -> p t", p=128)
            )
            nc.sync.dma_start_transpose(ctxT1[:], ctx_d[512:1024, :])

            # ---- Q projection unit: one (dc, qf) chain -> fp8 drain ----
            def q_unit(dc, qf):
                pq = ps_sc.tile([128, 3, 512], F32, tag="sc", name="pq")
                for c in range(CQ):
                    nc.tensor.matmul(
                        pq[:, 0, :],
                        wq[:, c, dc * 128 : (dc + 1) * 128],
                        xTh[qf][:, c, :],
                        start=(c == 0),
                        stop=(c == CQ - 1),
                    )
                nc.vector.tensor_scalar_mul(
                    qT8h[qf][:, dc, :], pq[:, 0, :], FP8_PRE
                )

            def q_fold(qf):
                for dc in range(CI):
                    for j in range(4):
                        nc.sync.dma_start(
                            out=qT8fh[qf][:, 2 * dc + j // 2, j % 2, :],
                            in_=qT8h[qf][32 * j : 32 * j + 32, dc, :],
                        )

            # ---- K/V production units (blocks 0,1 separate; B merged) ----
            def ctx_of(base):
                if base < 512:
                    return ctxT0, base
                if base < MA:
                    return ctxT1, base - 512
                return ctxTB, base - MA

            def k_unit(g, dc):
                g0, gn = BLOCKS[g]
                bw = gn * 128
                ctx_t, lo = ctx_of(g0 * 128)
                k_t, klo = (
                    (kT8b[g], 0) if g < 2 else (kT8B, g0 * 128 - MA)
                )
                pk = ps_kv.tile([128, 512], F32, tag="kv", name="pk")
                for c in range(CC):
                    nc.tensor.matmul(
                        pk[:, :bw],
                        wk[:, c, dc * 128 : (dc + 1) * 128],
                        ctx_t[:, c, lo : lo + bw],
                        start=(c == 0),
                        stop=(c == CC - 1),
                    )
                nc.vector.tensor_scalar_mul(
                    k_t[:, dc, klo : klo + bw], pk[:, :bw], FP8_PRE
                )

            def k_fold(part):
                k_t, k_f = (
                    (kT8b[part], kT8fb[part]) if part < 2 else (kT8B, kT8fB)
                )
                for dc in range(CI):
                    for j in range(4):
                        nc.sync.dma_start(
                            out=k_f[:, 2 * dc + j // 2, j % 2, :],
                            in_=k_t[32 * j : 32 * j + 32, dc, :],
                        )

            def v_unit(kt):
                ctx_t, lo = ctx_of(kt * 128)
                pv = ps_kv.tile([128, 512], F32, tag="kv", name="pv")
                for c in range(CC):
                    nc.tensor.matmul(
                        pv[:],
                        ctx_t[:, c, lo : lo + 128],
                        wv[:, c, :],
                        start=(c == 0),
                        stop=(c == CC - 1),
                    )
                v2h = v2t[kt]
                nc.vector.tensor_copy(
                    v2h[:, :, 0:64], pv[:].rearrange("p (h d) -> p h d", d=64)
                )
                nc.vector.tensor_copy(
                    v2h[:, :, 64:65], valid[:, kt : kt + 1].to_broadcast([128, H, 1])
                )

            # Up-front PE: Q units then K block 0; everything else deferred
            for dc in range(CI):
                q_unit(dc, 0)
                q_unit(dc, 1)
            for dc in range(CI):
                k_unit(0, dc)
            q_fold(0)
            q_fold(1)
            k_fold(0)
            nc.scalar.dma_start(
                out=wvo[:], in_=wvo_d[:].rearrange("(o p) f -> p o f", p=128)
            )
            nc.scalar.dma_start(
                out=bo_bc[:],
                in_=bass.AP(tensor=bo_d, offset=0, ap=[[0, 128], [1, QUERY_DIM]]),
            )
            nc.sync.dma_start_transpose(ctxTB[:], ctx_d[MA:m_pad, :])

            # pass-B-consumed production, emitted inside pass-A rounds r>=1
            deferred = []
            for g in range(BA, NB):
                for dc in range(CI):
                    deferred.append(lambda g=g, dc=dc: k_unit(g, dc))
                for kt in range(BLOCKS[g][0], BLOCKS[g][0] + BLOCKS[g][1]):
                    deferred.append(lambda kt=kt: v_unit(kt))
            deferred.append(lambda: k_fold(2))
            di = [0]

            def emit_units(k):
                while k > 0 and di[0] < len(deferred):
                    deferred[di[0]]()
                    di[0] += 1
                    k -= 1

            # ---- attention rounds ----
            def do_pass(qb, h, groups, upsum, interleave, weave=None,
                        dbg_cap=None):
                from concourse.instruction_name_ordered_set import (
                    InstructionNameOrderedSet,
                )
                first = True
                start_inst = None
                last_g = len(groups) - 1
                for gi, kts in enumerate(groups):
                    gn = len(kts)
                    sc = ps_sc.tile([128, 3, 512], F32, tag="sc", name="sc")
                    for j, kt in enumerate(kts):
                        base = kt * 128
                        if base < 512:
                            k_f, lo = kT8fb[0], base
                        elif base < MA:
                            k_f, lo = kT8fb[1], base - 512
                        else:
                            k_f, lo = kT8fB, base - MA
                        nc.tensor.matmul(
                            sc[:, j, :],
                            k_f[:, h, :, lo : lo + 128],
                            qT8fh[qb][:, h, :, :],
                            start=True,
                            stop=True,
                            perf_mode=PM.DoubleRow,
                        )
                    pt = ptp.tile([128, 3, 512], BF, tag="pt", name="pt")
                    nc.scalar.activation(
                        pt[:, 0:gn, :], sc[:, 0:gn, :], AF.Exp, scale=EXP_SCALE
                    )
                    if dbg_cap is not None and gi == 0:
                        dd = nc.declare_dram_parameter(
                            "d_pt0", [128, 3, 512], BF, isOutput=True
                        )
                        nc.sync.dma_start(out=dd[:], in_=pt[:])
                        dbg_cap.append(1)
                    if weave is not None:
                        for fn_ in weave.get(gi, ()):
                            fn_()
                    if interleave:
                        emit_units(2 if di[0] < 13 else 1)
                    for j, kt in enumerate(kts):
                        for qt in range(4):
                            nc.tensor.matmul(
                                upsum[:, qt, 0:65],
                                pt[:, j, qt * 128 : (qt + 1) * 128],
                                v2t[kt][:, h, :],
                                start=False,
                                stop=(gi == last_g) and (j == gn - 1),
                                skip_group_check=True,
                            )
                    first = False

            # pass A: U_a -> SBUF.  Round 0 weaves in K block 1 (+fold) and
            # the A-side V units just ahead of their first consumers.
            a_kts = GR_A
            weave0 = {}
            w0 = [lambda dc=dc: k_unit(1, dc) for dc in range(CI)]
            w0.append(lambda: k_fold(1))
            w0 += [lambda kt=kt: v_unit(kt) for kt in a_kts[0]]
            weave0[0] = w0
            for gi in range(1, len(a_kts)):
                weave0[gi] = [lambda kt=kt: v_unit(kt) for kt in a_kts[gi]]
            dbg_pt = []
            for r in range(16):
                qb, h = r // 8, r % 8
                ua_ps = ps_u.tile([128, 4, 128], F32, tag="u", name="ua_ps")
                nc.vector.memset(ua_ps[:], 0.0)
                do_pass(
                    qb, h, GR_A, ua_ps, r >= 1,
                    weave=weave0 if r == 0 else None,
                    dbg_cap=dbg_pt if (_DBG and r == 0) else None,
                )
                nc.vector.tensor_copy(ua[r][:], ua_ps[:, :, 0:65])

            # pass B: U_b + U_a -> normalize -> O; out-proj per query block
            def out_proj_qt(qb, qt):
                oT = otp.tile([128, CI, 128], BF, tag="oT", name="oT")
                nc.sync.dma_start_transpose(oT[:], onat[qb][:, qt, :])
                po = ps_sc.tile([128, 3, 512], F32, tag="sc", name="po")
                for c in range(CI):
                    nc.tensor.matmul(
                        po[:, 0, :],
                        oT[:, c, :],
                        wo[:, c, :],
                        start=(c == 0),
                        stop=(c == CI - 1),
                    )
                ot = outp.tile([128, QUERY_DIM], F32, tag="ot", name="ot")
                nc.vector.tensor_add(ot[:], po[:, 0, :], bo_bc[:])
                qrow = (qb * 4 + qt) * 128
                nc.sync.dma_start(out=out_d[qrow : qrow + 128, :], in_=ot[:])

            for r in range(16):
                qb, h = r // 8, r % 8
                ub_ps = ps_u.tile([128, 4, 128], F32, tag="u", name="ub_ps")
                nc.vector.memset(ub_ps[:], 0.0)
                do_pass(qb, h, GR_B, ub_ps, False)
                fin = finp.tile([128, 4, 65], F32, tag="fin", name="fin")
                nc.vector.tensor_add(fin[:], ub_ps[:, :, 0:65], ua[r][:])
                nc.vector.reciprocal(fin[:, :, 64:65], fin[:, :, 64:65])
                for qt in range(4):
                    nc.vector.tensor_scalar_mul(
                        onat[qb][:, qt, h * 64 : (h + 1) * 64],
                        fin[:, qt, 0:64],
                        fin[:, qt, 64:65],
                    )
                if r >= 9 and r % 2 == 1:
                    out_proj_qt(0, (r - 9) // 2)
            # final block: batch transposes, then matmul+add chains, then stores
            oTs = []
            for qt in range(4):
                oT = otp.tile([128, CI, 128], BF, tag="oTf", name=f"oTf{qt}")
                nc.sync.dma_start_transpose(oT[:], onat[1][:, qt, :])
                oTs.append(oT)
            ots = []
            for qt in range(4):
                po = ps_sc.tile([128, 3, 512], F32, tag="sc", name="pof")
                for c in range(CI):
                    nc.tensor.matmul(
                        po[:, 0, :],
                        oTs[qt][:, c, :],
                        wo[:, c, :],
                        start=(c == 0),
                        stop=(c == CI - 1),
                    )
                ot = outp.tile([128, QUERY_DIM], F32, tag="otf", name=f"otf{qt}")
                nc.vector.tensor_add(ot[:], po[:, 0, :], bo_bc[:])
                ots.append(ot)
            for qt in range(4):
                qrow = (4 + qt) * 128
                nc.sync.dma_start(out=out_d[qrow : qrow + 128, :], in_=ots[qt][:])

            if _DBG:
                dumps = [
                    ("d_qT8f0", qT8fh[0], [32, H, 2, 512]),
                    ("d_kT8fb0", kT8fb[0], [32, H, 2, 512]),
                    ("d_kT8b0", kT8b[0], [128, CI, 512]),
                    ("d_v2_0", v2t[0], [128, H, 65]),
                    ("d_ua0", ua[0], [128, 4, 65]),
                    ("d_on0", onat[0], [128, 4, INNER]),
                    ("d_xT0", xTh[0], [128, CQ, 512]),
                    ("d_cT0", ctxT0, [128, CC, 512]),
                ]
                for nm, t, shp in dumps:
                    dt_ = t[:].dtype
                    dd = nc.declare_dram_parameter(nm, shp, dt_, isOutput=True)
                    nc.sync.dma_start(out=dd[:], in_=t[:])

    nc.compile()
    return nc


def kernel(x, context_tensor, mask, Wq, Wk, Wv, Wo, bo):
    from concourse.bass_utils import run_bass_kernel_spmd

    x = np.asarray(x, dtype=np.float32)
    context_tensor = np.asarray(context_tensor, dtype=np.float32)
    mask = np.asarray(mask)
    Wq = np.asarray(Wq, dtype=np.float32)
    Wk = np.asarray(Wk, dtype=np.float32)
    Wv = np.asarray(Wv, dtype=np.float32)
    Wo = np.asarray(Wo, dtype=np.float32)
    bo = np.ascontiguousarray(np.asarray(bo, dtype=np.float32))

    # host-side context compaction using the mask
    meffs = [int(mask[b].sum()) for b in range(B)]
    m_pad = max(M_PAD_MIN, ((max(meffs) + 127) // 128) * 128)
    ctx_c = np.zeros((B, m_pad, CONTEXT_DIM), dtype=BF16)
    val = np.zeros((B, m_pad), dtype=BF16)
    for b in range(B):
        idx = np.flatnonzero(mask[b])
        ctx_c[b, : len(idx)] = context_tensor[b, idx].astype(BF16)
        val[b, : len(idx)] = 1.0

    if m_pad not in _compiled:
        _compiled[m_pad] = _build(m_pad)
    nc = _compiled[m_pad]

    x16 = np.ascontiguousarray(x.astype(BF16))
    wqk16 = np.ascontiguousarray(np.concatenate([Wq, Wk], axis=0).astype(BF16))
    wvo16 = np.ascontiguousarray(np.concatenate([Wv, Wo], axis=0).astype(BF16))

    rows_per_core = N // (NCORES // B)  # 1024
    in_maps = []
    for d in range(NCORES):
        b = d // (NCORES // B)
        r0 = (d % (NCORES // B)) * rows_per_core
        in_maps.append(
            {
                "xs": np.ascontiguousarray(x16[b, r0 : r0 + rows_per_core]),
                "ctx": ctx_c[b],
                "valid": val[b],
                "Wqk": wqk16,
                "Wvo": wvo16,
                "bo": bo,
            }
        )

    res = run_bass_kernel_spmd(nc, in_maps, list(range(NCORES)))
    global _last_results
    _last_results = res
    out = np.empty((B, N, QUERY_DIM), dtype=np.float32)
    for d in range(NCORES):
        b = d // (NCORES // B)
        r0 = (d % (NCORES // B)) * rows_per_core
        out[b, r0 : r0 + rows_per_core] = res.results[d]["out"]
    return out


# revision 28
# speedup vs baseline: 1.1981x; 1.1295x over previous
"""Cross-attention Bass kernel for 8 trn2 NeuronCores.

Sharding: core d handles batch b = d//4 and query rows [(d%4)*1024, (d%4+1)*1024)
of that batch, computing all 8 heads (no collectives). The context is compacted
on the host using the mask (masked rows dropped, zero-padded to a multiple of
128), which preserves softmax semantics exactly.

Design (v3):
- All data bf16 except: PSUM (f32), scores path (fp8e4 DoubleRow).
- x^T / ctx^T produced by XBAR DMA-transpose straight out of DRAM (inputs are
  host-cast to bf16); weight tiles use the matching (o p) f -> p o f layout.
- Scores: Q^T/K^T drained to fp8 (x16) and partition-folded via small
  SBUF->SBUF DMAs into [32, (h, 2, m)] so one DoubleRow matmul per (head,
  k-tile) contracts all 64 head dims at 0.5 cycles/row.
- P = exp(S) on ACT straight out of 3-bank PSUM score groups into bf16; the
  softmax scale and fp8 prescale fold into the exp. ACT runs ONLY exps.
- PV uses P^T chunks as stationary and V (+ones column for the denominator)
  as the 65-wide moving operand. U accumulates per (qtile,head) in one PSUM
  bank (4 slots, first-write start marks the zero region).
- Two passes over k-groups (A: first 8 k-tiles, B: rest) bound U-psum
  residency, so pass-A rounds start after two context blocks; blocks 2+ are
  produced inside pass-A rounds to keep PE busy under the ACT exp stream.
- Normalization is a per-partition broadcast multiply on DVE (q is partition
  dim after PV); O is DMA-transposed for the output projection.
- Every tile is written-once-read-later at whole-tile granularity (per-block
  / per-half tiles) to avoid false WAR/RAW serialization in Tile's tracker.
"""
import numpy as np
import ml_dtypes

B, N, M = 2, 4096, 4096
QUERY_DIM, CONTEXT_DIM = 512, 768
H, D = 8, 64
INNER = H * D  # 512
NCORES = 8
N_DEV = (B * N) // NCORES  # 1024 query rows per core
M_PAD_MIN = 1792

BF16 = ml_dtypes.bfloat16
SCALE = float(D) ** -0.5
FP8_PRE = 16.0  # fp8 prescale on Q and K; exp scale divides by 16*16

_compiled = {}
_DBG = False


def _build(m_pad):
    from concourse import bacc
    import concourse.bass as bass
    import concourse.mybir as mybir
    import concourse.tile as tile

    F32 = mybir.dt.float32
    BF = mybir.dt.bfloat16
    F8 = mybir.dt.float8e4
    AF = mybir.ActivationFunctionType
    PM = mybir.MatmulPerfMode

    KT = m_pad // 128  # k-tiles (17 for 2176)
    # context blocks of up to 4 k-tiles
    BLOCKS = [(4 * g, min(4, KT - 4 * g)) for g in range((KT + 3) // 4)]
    NB = len(BLOCKS)
    BA = 2  # pass A covers blocks [0, BA), pass B the rest
    CQ = QUERY_DIM // 128  # 4
    CC = CONTEXT_DIM // 128  # 6
    CI = INNER // 128  # 4
    EXP_SCALE = SCALE / (FP8_PRE * FP8_PRE)

    # exp groups of up to 3 k-tiles within each pass (psum: 3 banks x 2 bufs)
    def make_groups(kts):
        out = []
        i = 0
        while i < len(kts):
            out.append(kts[i : i + 3])
            i += 3
        return out

    A_KT = [kt for g0, gn in BLOCKS[:BA] for kt in range(g0, g0 + gn)]
    B_KT = [kt for g0, gn in BLOCKS[BA:] for kt in range(g0, g0 + gn)]
    GR_A = make_groups(A_KT)  # e.g. [[0,1,2],[3,4,5],[6,7]]
    GR_B = make_groups(B_KT)  # e.g. [[8,9,10],[11,12,13],[14,15,16]]

    nc = bacc.Bacc()
    xs_d = nc.declare_dram_parameter("xs", [N_DEV, QUERY_DIM], BF, isOutput=False)
    ctx_d = nc.declare_dram_parameter("ctx", [m_pad, CONTEXT_DIM], BF, isOutput=False)
    val_d = nc.declare_dram_parameter("valid", [m_pad], BF, isOutput=False)
    wqk_d = nc.declare_dram_parameter(
        "Wqk", [QUERY_DIM + CONTEXT_DIM, INNER], BF, isOutput=False
    )
    wvo_d = nc.declare_dram_parameter(
        "Wvo", [CONTEXT_DIM + INNER, INNER], BF, isOutput=False
    )
    bo_d = nc.declare_dram_parameter("bo", [QUERY_DIM], F32, isOutput=False)
    out_d = nc.declare_dram_parameter("out", [N_DEV, QUERY_DIM], F32, isOutput=True)

    with tile.TileContext(nc) as tc:
        with (
            tc.tile_pool(name="big", bufs=1) as big,
            tc.tile_pool(name="pt", bufs=2) as ptp,
            tc.tile_pool(name="fin", bufs=2) as finp,
            tc.tile_pool(name="otp", bufs=2) as otp,
            tc.tile_pool(name="outp", bufs=2) as outp,
            tc.tile_pool(name="ps_sc", bufs=2, space="PSUM") as ps_sc,
            tc.tile_pool(name="ps_u", bufs=1, space="PSUM") as ps_u,
            tc.tile_pool(name="ps_kv", bufs=1, space="PSUM") as ps_kv,
        ):
            # ---- static SBUF tiles (each written once, at one spot) ----
            wqk = big.tile([128, CQ + CC, INNER], BF, tag="wqk", name="wqk")
            wvo = big.tile([128, CC + CI, INNER], BF, tag="wvo", name="wvo")
            wq = wqk[:, 0:CQ, :]
            wk = wqk[:, CQ : CQ + CC, :]
            wv = wvo[:, 0:CC, :]
            wo = wvo[:, CC : CC + CI, :]
            bo_bc = big.tile([128, QUERY_DIM], F32, tag="bo", name="bo")
            valid = big.tile([128, KT], BF, tag="valid", name="valid")
            xTh = [
                big.tile([128, CQ, 512], BF, tag=f"xT{i}", name=f"xT{i}")
                for i in range(2)
            ]
            MA = min(1024, m_pad)  # pass-A context columns
            MB = m_pad - MA
            ctxT0 = big.tile([128, CC, 512], BF, tag="cT0", name="cT0")
            ctxT1 = big.tile([128, CC, 512], BF, tag="cT1", name="cT1")
            ctxTB = big.tile([128, CC, MB], BF, tag="cTB", name="cTB")
            qT8h = [
                big.tile([128, CI, 512], F8, tag=f"qT8_{i}", name=f"qT8_{i}")
                for i in range(2)
            ]
            qT8fh = [
                big.tile([32, H, 2, 512], F8, tag=f"qT8f_{i}", name=f"qT8f_{i}")
                for i in range(2)
            ]
            kT8b = [
                big.tile([128, CI, 512], F8, tag=f"kT8b{i}", name=f"kT8b{i}")
                for i in range(2)
            ]
            kT8B = big.tile([128, CI, MB], F8, tag="kT8B", name="kT8B")
            kT8fb = [
                big.tile([32, H, 2, 512], F8, tag=f"kfb{i}", name=f"kfb{i}")
                for i in range(2)
            ]
            kT8fB = big.tile([32, H, 2, MB], F8, tag="kfB", name="kfB")
            v2t = [
                big.tile([128, H, 65], BF, tag=f"v2_{t}", name=f"v2_{t}")
                for t in range(KT)
            ]
            ua = [
                big.tile([128, 4, 65], F32, tag=f"ua{r}", name=f"ua{r}")
                for r in range(16)
            ]
            onat = [
                big.tile([128, 4, INNER], BF, tag=f"on{qb}", name=f"on{qb}")
                for qb in range(2)
            ]

            # ---- loads + input transposes (DMA device is serialized: Q path
            # first, then ctx blocks in consumption order) ----
            # Few, large DMAs: chains between unlike DMAs then cost little.
            nc.sync.dma_start_transpose(xTh[0][:], xs_d[0:512, :])
            nc.sync.dma_start_transpose(xTh[1][:], xs_d[512:1024, :])
            nc.sync.dma_start_transpose(ctxT0[:], ctx_d[0:512, :])
            nc.scalar.dma_start(
                out=wqk[:], in_=wqk_d[:].rearrange("(o p) f -> p o f", p=128)
            )
            nc.scalar.dma_start(
                out=valid[:], in_=val_d[:].rearrange("(t p) -> p t", p=128)
            )
            nc.sync.dma_start_transpose(ctxT1[:], ctx_d[512:1024, :])

            # rotating psum slots for the prologue projection chains: the
            # two 3-bank sc tiles provide 6 independent banks
            _slot_state = {"tile": None, "i": 0}

            def next_slot():
                if _slot_state["i"] % 3 == 0:
                    _slot_state["tile"] = ps_sc.tile(
                        [128, 3, 512], F32, tag="sc", name="slot"
                    )
                j = _slot_state["i"] % 3
                _slot_state["i"] += 1
                return _slot_state["tile"][:, j, :]

            # ---- Q projection unit: one (dc, qf) chain -> fp8 drain ----
            def q_unit(dc, qf):
                pq = next_slot()
                for c in range(CQ):
                    nc.tensor.matmul(
                        pq,
                        wq[:, c, dc * 128 : (dc + 1) * 128],
                        xTh[qf][:, c, :],
                        start=(c == 0),
                        stop=(c == CQ - 1),
                    )
                nc.vector.tensor_scalar_mul(qT8h[qf][:, dc, :], pq, FP8_PRE)

            def q_fold(qf):
                # one DMA per 32-partition quarter: all 4 dc chunks at once
                # (dst heads h = 2*dc + j//2 are stride-2 in h)
                dst5 = qT8fh[qf][:].rearrange(
                    "p (hh hl) i m -> p hl hh i m", hl=2
                )
                for j in range(4):
                    nc.sync.dma_start(
                        out=dst5[:, j // 2, :, j % 2, :],
                        in_=qT8h[qf][32 * j : 32 * j + 32, :, :],
                    )

            # ---- K/V production units (blocks 0,1 separate; B merged) ----
            def ctx_of(base):
                if base < 512:
                    return ctxT0, base
                if base < MA:
                    return ctxT1, base - 512
                return ctxTB, base - MA

            def k_unit(g, dc, use_slot=False):
                g0, gn = BLOCKS[g]
                bw = gn * 128
                ctx_t, lo = ctx_of(g0 * 128)
                k_t, klo = (
                    (kT8b[g], 0) if g < 2 else (kT8B, g0 * 128 - MA)
                )
                pk = (
                    next_slot()
                    if use_slot
                    else ps_kv.tile([128, 512], F32, tag="kv", name="pk")[:]
                )
                for c in range(CC):
                    nc.tensor.matmul(
                        pk[:, 0:bw],
                        wk[:, c, dc * 128 : (dc + 1) * 128],
                        ctx_t[:, c, lo : lo + bw],
                        start=(c == 0),
                        stop=(c == CC - 1),
                    )
                nc.vector.tensor_scalar_mul(
                    k_t[:, dc, klo : klo + bw], pk[:, 0:bw], FP8_PRE
                )

            def k_fold(part):
                k_t, k_f = (
                    (kT8b[part], kT8fb[part]) if part < 2 else (kT8B, kT8fB)
                )
                dst5 = k_f[:].rearrange("p (hh hl) i m -> p hl hh i m", hl=2)
                for j in range(4):
                    nc.sync.dma_start(
                        out=dst5[:, j // 2, :, j % 2, :],
                        in_=k_t[32 * j : 32 * j + 32, :, :],
                    )

            def v_unit(kt):
                ctx_t, lo = ctx_of(kt * 128)
                pv = ps_kv.tile([128, 512], F32, tag="kv", name="pv")
                for c in range(CC):
                    nc.tensor.matmul(
                        pv[:],
                        ctx_t[:, c, lo : lo + 128],
                        wv[:, c, :],
                        start=(c == 0),
                        stop=(c == CC - 1),
                    )
                v2h = v2t[kt]
                nc.vector.tensor_copy(
                    v2h[:, :, 0:64], pv[:].rearrange("p (h d) -> p h d", d=64)
                )
                nc.vector.tensor_copy(
                    v2h[:, :, 64:65], valid[:, kt : kt + 1].to_broadcast([128, H, 1])
                )

            # Up-front PE: Q chains interleaved with K blocks 0-1 over the
            # six rotating sc banks, so PE stays dense and fully ramped
            qlist = [(dc, qf) for dc in range(CI) for qf in range(2)]
            for i, (dc, qf) in enumerate(qlist):
                q_unit(dc, qf)
                if i >= 4:
                    k_unit(0, i - 4, use_slot=True)
            q_fold(0)
            q_fold(1)
            k_fold(0)
            nc.gpsimd.dma_start(
                out=wvo[:], in_=wvo_d[:].rearrange("(o p) f -> p o f", p=128)
            )
            nc.gpsimd.dma_start(
                out=bo_bc[:],
                in_=bass.AP(tensor=bo_d, offset=0, ap=[[0, 128], [1, QUERY_DIM]]),
            )
            nc.sync.dma_start_transpose(ctxTB[:], ctx_d[MA:m_pad, :])

            # pass-B-consumed production, emitted inside pass-A rounds r>=1
            deferred = []
            for g in range(BA, NB):
                for dc in range(CI):
                    deferred.append(lambda g=g, dc=dc: k_unit(g, dc))
                for kt in range(BLOCKS[g][0], BLOCKS[g][0] + BLOCKS[g][1]):
                    deferred.append(lambda kt=kt: v_unit(kt))
            deferred.append(lambda: k_fold(2))
            di = [0]

            def emit_units(k):
                while k > 0 and di[0] < len(deferred):
                    deferred[di[0]]()
                    di[0] += 1
                    k -= 1

            # ---- attention rounds ----
            def do_pass(qb, h, groups, upsum, interleave, weave=None,
                        dbg_cap=None):
                from concourse.instruction_name_ordered_set import (
                    InstructionNameOrderedSet,
                )
                first = True
                start_inst = None
                last_g = len(groups) - 1
                for gi, kts in enumerate(groups):
                    gn = len(kts)
                    sc = ps_sc.tile([128, 3, 512], F32, tag="sc", name="sc")
                    for j, kt in enumerate(kts):
                        base = kt * 128
                        if base < 512:
                            k_f, lo = kT8fb[0], base
                        elif base < MA:
                            k_f, lo = kT8fb[1], base - 512
                        else:
                            k_f, lo = kT8fB, base - MA
                        nc.tensor.matmul(
                            sc[:, j, :],
                            k_f[:, h, :, lo : lo + 128],
                            qT8fh[qb][:, h, :, :],
                            start=True,
                            stop=True,
                            perf_mode=PM.DoubleRow,
                        )
                    pt = ptp.tile([128, 3, 512], BF, tag="pt", name="pt")
                    nc.scalar.activation(
                        pt[:, 0:gn, :], sc[:, 0:gn, :], AF.Exp, scale=EXP_SCALE
                    )
                    if dbg_cap is not None and gi == 0:
                        dd = nc.declare_dram_parameter(
                            "d_pt0", [128, 3, 512], BF, isOutput=True
                        )
                        nc.sync.dma_start(out=dd[:], in_=pt[:])
                        dbg_cap.append(1)
                    if weave is not None:
                        with tc.high_priority(offset=-1000000):
                            for fn_ in weave.get(gi, ()):
                                fn_()
                    if interleave:
                        with tc.high_priority(offset=-1000000):
                            emit_units(2 if di[0] < 13 else 1)
                    for j, kt in enumerate(kts):
                        for qt in range(4):
                            nc.tensor.matmul(
                                upsum[:, qt, 0:65],
                                pt[:, j, qt * 128 : (qt + 1) * 128],
                                v2t[kt][:, h, :],
                                start=False,
                                stop=(gi == last_g) and (j == gn - 1),
                                skip_group_check=True,
                            )
                    first = False

            # pass A: U_a -> SBUF.  Round 0 weaves in K block 1 (+fold) and
            # the A-side V units just ahead of their first consumers.
            a_kts = GR_A
            weave0 = {
                gi: [lambda kt=kt: v_unit(kt) for kt in a_kts[gi]]
                for gi in range(len(a_kts))
            }
            w0 = [lambda dc=dc: k_unit(1, dc) for dc in range(CI)]
            w0.append(lambda: k_fold(1))
            weave0[0] = w0 + weave0[0]
            dbg_pt = []
            for r in range(16):
                qb, h = r // 8, r % 8
                ua_ps = ps_u.tile([128, 4, 128], F32, tag="u", name="ua_ps")
                nc.vector.memset(ua_ps[:], 0.0)
                do_pass(
                    qb, h, GR_A, ua_ps, r >= 1,
                    weave=weave0 if r == 0 else None,
                    dbg_cap=dbg_pt if (_DBG and r == 0) else None,
                )
                nc.vector.tensor_copy(ua[r][:], ua_ps[:, :, 0:65])

            # pass B: U_b + U_a -> normalize -> O; out-proj per query block
            def out_proj_qt(qb, qt):
                oT = otp.tile([128, CI, 128], BF, tag="oT", name="oT")
                nc.sync.dma_start_transpose(oT[:], onat[qb][:, qt, :])
                po = ps_kv.tile([128, 512], F32, tag="kv", name="po")
                for c in range(CI):
                    nc.tensor.matmul(
                        po[:],
                        oT[:, c, :],
                        wo[:, c, :],
                        start=(c == 0),
                        stop=(c == CI - 1),
                    )
                ot = outp.tile([128, QUERY_DIM], F32, tag="ot", name="ot")
                nc.vector.tensor_add(ot[:], po[:], bo_bc[:])
                qrow = (qb * 4 + qt) * 128
                nc.sync.dma_start(out=out_d[qrow : qrow + 128, :], in_=ot[:])

            for r in range(16):
                qb, h = r // 8, r % 8
                ub_ps = ps_u.tile([128, 4, 128], F32, tag="u", name="ub_ps")
                nc.vector.memset(ub_ps[:], 0.0)
                do_pass(qb, h, GR_B, ub_ps, False)
                fin = finp.tile([128, 4, 65], F32, tag="fin", name="fin")
                nc.vector.tensor_add(fin[:], ub_ps[:, :, 0:65], ua[r][:])
                nc.vector.reciprocal(fin[:, :, 64:65], fin[:, :, 64:65])
                for qt in range(4):
                    nc.vector.tensor_scalar_mul(
                        onat[qb][:, qt, h * 64 : (h + 1) * 64],
                        fin[:, qt, 0:64],
                        fin[:, qt, 64:65],
                    )
                if r >= 9 and r % 2 == 1:
                    out_proj_qt(0, (r - 9) // 2)
            # final block: batch transposes, then matmul+add chains, then stores
            oTs = []
            for qt in range(4):
                oT = otp.tile([128, CI, 128], BF, tag="oTf", name=f"oTf{qt}")
                nc.sync.dma_start_transpose(oT[:], onat[1][:, qt, :])
                oTs.append(oT)
            otf = big.tile([128, 4, QUERY_DIM], F32, tag="otf", name="otf")
            for qt in range(4):
                po = ps_kv.tile([128, 512], F32, tag="kv", name="pof")
                for c in range(CI):
                    nc.tensor.matmul(
                        po[:],
                        oTs[qt][:, c, :],
                        wo[:, c, :],
                        start=(c == 0),
                        stop=(c == CI - 1),
                    )
                nc.vector.tensor_add(otf[:, qt, :], po[:], bo_bc[:])
            nc.sync.dma_start(
                out=out_d[512:1024, :].rearrange("(t p) f -> p t f", p=128),
                in_=otf[:],
            )

            if _DBG:
                dumps = [
                    ("d_qT8f0", qT8fh[0], [32, H, 2, 512]),
                    ("d_kT8fb0", kT8fb[0], [32, H, 2, 512]),
                    ("d_kT8b0", kT8b[0], [128, CI, 512]),
                    ("d_v2_0", v2t[0], [128, H, 65]),
                    ("d_ua0", ua[0], [128, 4, 65]),
                    ("d_on0", onat[0], [128, 4, INNER]),
                    ("d_xT0", xTh[0], [128, CQ, 512]),
                    ("d_cT0", ctxT0, [128, CC, 512]),
                ]
                for nm, t, shp in dumps:
                    dt_ = t[:].dtype
                    dd = nc.declare_dram_parameter(nm, shp, dt_, isOutput=True)
                    nc.sync.dma_start(out=dd[:], in_=t[:])

    nc.compile()
    return nc


def kernel(x, context_tensor, mask, Wq, Wk, Wv, Wo, bo):
    from concourse.bass_utils import run_bass_kernel_spmd

    x = np.asarray(x, dtype=np.float32)
    context_tensor = np.asarray(context_tensor, dtype=np.float32)
    mask = np.asarray(mask)
    Wq = np.asarray(Wq, dtype=np.float32)
    Wk = np.asarray(Wk, dtype=np.float32)
    Wv = np.asarray(Wv, dtype=np.float32)
    Wo = np.asarray(Wo, dtype=np.float32)
    bo = np.ascontiguousarray(np.asarray(bo, dtype=np.float32))

    # host-side context compaction using the mask
    meffs = [int(mask[b].sum()) for b in range(B)]
    m_pad = max(M_PAD_MIN, ((max(meffs) + 127) // 128) * 128)
    ctx_c = np.zeros((B, m_pad, CONTEXT_DIM), dtype=BF16)
    val = np.zeros((B, m_pad), dtype=BF16)
    for b in range(B):
        idx = np.flatnonzero(mask[b])
        ctx_c[b, : len(idx)] = context_tensor[b, idx].astype(BF16)
        val[b, : len(idx)] = 1.0

    if m_pad not in _compiled:
        _compiled[m_pad] = _build(m_pad)
    nc = _compiled[m_pad]

    x16 = np.ascontiguousarray(x.astype(BF16))
    wqk16 = np.ascontiguousarray(np.concatenate([Wq, Wk], axis=0).astype(BF16))
    wvo16 = np.ascontiguousarray(np.concatenate([Wv, Wo], axis=0).astype(BF16))

    rows_per_core = N // (NCORES // B)  # 1024
    in_maps = []
    for d in range(NCORES):
        b = d // (NCORES // B)
        r0 = (d % (NCORES // B)) * rows_per_core
        in_maps.append(
            {
                "xs": np.ascontiguousarray(x16[b, r0 : r0 + rows_per_core]),
                "ctx": ctx_c[b],
                "valid": val[b],
                "Wqk": wqk16,
                "Wvo": wvo16,
                "bo": bo,
            }
        )

    res = run_bass_kernel_spmd(nc, in_maps, list(range(NCORES)))
    global _last_results
    _last_results = res
    out = np.empty((B, N, QUERY_DIM), dtype=np.float32)
    for d in range(NCORES):
        b = d // (NCORES // B)
        r0 = (d % (NCORES // B)) * rows_per_core
        out[b, r0 : r0 + rows_per_core] = res.results[d]["out"]
    return out


# revision 36
# speedup vs baseline: 1.2555x; 1.0479x over previous
"""Cross-attention Bass kernel for 8 trn2 NeuronCores.

Sharding: core d handles batch b = d//4 and query rows [(d%4)*1024, (d%4+1)*1024)
of that batch, computing all 8 heads (no collectives). The context is compacted
on the host using the mask (masked rows dropped, zero-padded to a multiple of
128), which preserves softmax semantics exactly.

Design (v3):
- All data bf16 except: PSUM (f32), scores path (fp8e4 DoubleRow).
- x^T / ctx^T produced by XBAR DMA-transpose straight out of DRAM (inputs are
  host-cast to bf16); weight tiles use the matching (o p) f -> p o f layout.
- Scores: Q^T/K^T drained to fp8 (x16) and partition-folded via small
  SBUF->SBUF DMAs into [32, (h, 2, m)] so one DoubleRow matmul per (head,
  k-tile) contracts all 64 head dims at 0.5 cycles/row.
- P = exp(S) on ACT straight out of 3-bank PSUM score groups into bf16; the
  softmax scale and fp8 prescale fold into the exp. ACT runs ONLY exps.
- PV uses P^T chunks as stationary and V (+ones column for the denominator)
  as the 65-wide moving operand. U accumulates per (qtile,head) in one PSUM
  bank (4 slots, first-write start marks the zero region).
- Two passes over k-groups (A: first 8 k-tiles, B: rest) bound U-psum
  residency, so pass-A rounds start after two context blocks; blocks 2+ are
  produced inside pass-A rounds to keep PE busy under the ACT exp stream.
- Normalization is a per-partition broadcast multiply on DVE (q is partition
  dim after PV); O is DMA-transposed for the output projection.
- Every tile is written-once-read-later at whole-tile granularity (per-block
  / per-half tiles) to avoid false WAR/RAW serialization in Tile's tracker.
"""
import numpy as np
import ml_dtypes

B, N, M = 2, 4096, 4096
QUERY_DIM, CONTEXT_DIM = 512, 768
H, D = 8, 64
INNER = H * D  # 512
NCORES = 8
N_DEV = (B * N) // NCORES  # 1024 query rows per core
M_PAD_MIN = 1792

BF16 = ml_dtypes.bfloat16
SCALE = float(D) ** -0.5
FP8_PRE = 16.0  # fp8 prescale on Q and K; exp scale divides by 16*16

_compiled = {}
_DBG = False


def _build(m_pad):
    from concourse import bacc
    import concourse.bass as bass
    import concourse.mybir as mybir
    import concourse.tile as tile

    F32 = mybir.dt.float32
    BF = mybir.dt.bfloat16
    F8 = mybir.dt.float8e4
    AF = mybir.ActivationFunctionType
    PM = mybir.MatmulPerfMode

    KT = m_pad // 128  # k-tiles (17 for 2176)
    # context blocks of up to 4 k-tiles
    BLOCKS = [(4 * g, min(4, KT - 4 * g)) for g in range((KT + 3) // 4)]
    NB = len(BLOCKS)
    BA = 2  # pass A covers blocks [0, BA), pass B the rest
    CQ = QUERY_DIM // 128  # 4
    CC = CONTEXT_DIM // 128  # 6
    CI = INNER // 128  # 4
    EXP_SCALE = SCALE / (FP8_PRE * FP8_PRE)

    # exp groups of up to 3 k-tiles within each pass (psum: 3 banks x 2 bufs)
    def make_groups(kts):
        out = []
        i = 0
        while i < len(kts):
            out.append(kts[i : i + 3])
            i += 3
        return out

    A_KT = [kt for g0, gn in BLOCKS[:BA] for kt in range(g0, g0 + gn)]
    B_KT = [kt for g0, gn in BLOCKS[BA:] for kt in range(g0, g0 + gn)]
    GR_A = make_groups(A_KT)  # e.g. [[0,1,2],[3,4,5],[6,7]]
    GR_B = make_groups(B_KT)  # e.g. [[8,9,10],[11,12,13],[14,15,16]]

    nc = bacc.Bacc()
    xs_d = nc.declare_dram_parameter("xs", [N_DEV, QUERY_DIM], BF, isOutput=False)
    ctx_d = nc.declare_dram_parameter("ctx", [m_pad, CONTEXT_DIM], BF, isOutput=False)
    val_d = nc.declare_dram_parameter("valid", [m_pad], BF, isOutput=False)
    wqk_d = nc.declare_dram_parameter(
        "Wqk", [QUERY_DIM + CONTEXT_DIM, INNER], BF, isOutput=False
    )
    wvo_d = nc.declare_dram_parameter(
        "Wvo", [CONTEXT_DIM + INNER, INNER], BF, isOutput=False
    )
    bo_d = nc.declare_dram_parameter("bo", [QUERY_DIM], F32, isOutput=False)
    out_d = nc.declare_dram_parameter("out", [N_DEV, QUERY_DIM], F32, isOutput=True)

    with tile.TileContext(nc) as tc:
        with (
            tc.tile_pool(name="big", bufs=1) as big,
            tc.tile_pool(name="pt", bufs=3) as ptp,
            tc.tile_pool(name="fin", bufs=2) as finp,
            tc.tile_pool(name="otp", bufs=2) as otp,
            tc.tile_pool(name="outp", bufs=2) as outp,
            tc.tile_pool(name="ps_sc", bufs=2, space="PSUM") as ps_sc,
            tc.tile_pool(name="ps_u", bufs=1, space="PSUM") as ps_u,
            tc.tile_pool(name="ps_kv", bufs=1, space="PSUM") as ps_kv,
        ):
            # ---- static SBUF tiles (each written once, at one spot) ----
            wqk = big.tile([128, CQ + CC, INNER], BF, tag="wqk", name="wqk")
            wvo = big.tile([128, CC + CI, INNER], BF, tag="wvo", name="wvo")
            wq = wqk[:, 0:CQ, :]
            wk = wqk[:, CQ : CQ + CC, :]
            wv = wvo[:, 0:CC, :]
            wo = wvo[:, CC : CC + CI, :]
            bo_bc = big.tile([128, QUERY_DIM], F32, tag="bo", name="bo")
            valid = big.tile([128, KT], BF, tag="valid", name="valid")
            xTh = [
                big.tile([128, CQ, 512], BF, tag=f"xT{i}", name=f"xT{i}")
                for i in range(2)
            ]
            MA = min(1024, m_pad)  # pass-A context columns
            MB = m_pad - MA
            ctxT0 = big.tile([128, CC, 512], BF, tag="cT0", name="cT0")
            ctxT1 = big.tile([128, CC, 512], BF, tag="cT1", name="cT1")
            ctxTB = big.tile([128, CC, MB], BF, tag="cTB", name="cTB")
            qT8h = [
                big.tile([128, CI, 512], F8, tag=f"qT8_{i}", name=f"qT8_{i}")
                for i in range(2)
            ]
            qT8fh = [
                big.tile([32, H, 2, 512], F8, tag=f"qT8f_{i}", name=f"qT8f_{i}")
                for i in range(2)
            ]
            kT8b = [
                big.tile([128, CI, 512], F8, tag=f"kT8b{i}", name=f"kT8b{i}")
                for i in range(2)
            ]
            kT8B = big.tile([128, CI, MB], F8, tag="kT8B", name="kT8B")
            kT8fb = [
                big.tile([32, H, 2, 512], F8, tag=f"kfb{i}", name=f"kfb{i}")
                for i in range(2)
            ]
            kT8fB = big.tile([32, H, 2, MB], F8, tag="kfB", name="kfB")
            v2t = [
                big.tile([128, H, 65], BF, tag=f"v2_{t}", name=f"v2_{t}")
                for t in range(KT)
            ]
            ua = [
                big.tile([128, 4, 65], F32, tag=f"ua{r}", name=f"ua{r}")
                for r in range(16)
            ]
            onat = [
                big.tile([128, 4, INNER], BF, tag=f"on{qb}", name=f"on{qb}")
                for qb in range(2)
            ]

            # ---- loads + input transposes (DMA device is serialized: Q path
            # first, then ctx blocks in consumption order) ----
            # Few, large DMAs: chains between unlike DMAs then cost little.
            nc.sync.dma_start_transpose(xTh[0][:], xs_d[0:512, :])
            nc.sync.dma_start_transpose(xTh[1][:], xs_d[512:1024, :])
            nc.sync.dma_start_transpose(ctxT0[:], ctx_d[0:512, :])
            nc.scalar.dma_start(
                out=wqk[:], in_=wqk_d[:].rearrange("(o p) f -> p o f", p=128)
            )
            nc.scalar.dma_start(
                out=valid[:], in_=val_d[:].rearrange("(t p) -> p t", p=128)
            )
            nc.sync.dma_start_transpose(ctxT1[:], ctx_d[512:1024, :])

            # rotating psum slots for the prologue projection chains: the
            # two 3-bank sc tiles provide 6 independent banks
            _slot_state = {"tile": None, "i": 0}

            def next_slot():
                if _slot_state["i"] % 3 == 0:
                    _slot_state["tile"] = ps_sc.tile(
                        [128, 3, 512], F32, tag="sc", name="slot"
                    )
                j = _slot_state["i"] % 3
                _slot_state["i"] += 1
                return _slot_state["tile"][:, j, :]

            # ---- Q projection unit: one (dc, qf) chain -> fp8 drain ----
            def q_unit(dc, qf):
                pq = next_slot()
                for c in range(CQ):
                    nc.tensor.matmul(
                        pq,
                        wq[:, c, dc * 128 : (dc + 1) * 128],
                        xTh[qf][:, c, :],
                        start=(c == 0),
                        stop=(c == CQ - 1),
                    )
                nc.vector.tensor_scalar_mul(qT8h[qf][:, dc, :], pq, FP8_PRE)

            def q_fold(qf):
                # one DMA per 32-partition quarter: all 4 dc chunks at once
                # (dst heads h = 2*dc + j//2 are stride-2 in h)
                dst5 = qT8fh[qf][:].rearrange(
                    "p (hh hl) i m -> p hl hh i m", hl=2
                )
                for j in range(4):
                    nc.sync.dma_start(
                        out=dst5[:, j // 2, :, j % 2, :],
                        in_=qT8h[qf][32 * j : 32 * j + 32, :, :],
                    )

            # ---- K/V production units (blocks 0,1 separate; B merged) ----
            def ctx_of(base):
                if base < 512:
                    return ctxT0, base
                if base < MA:
                    return ctxT1, base - 512
                return ctxTB, base - MA

            def k_unit(g, dc, use_slot=False):
                g0, gn = BLOCKS[g]
                bw = gn * 128
                ctx_t, lo = ctx_of(g0 * 128)
                k_t, klo = (
                    (kT8b[g], 0) if g < 2 else (kT8B, g0 * 128 - MA)
                )
                pk = (
                    next_slot()
                    if use_slot
                    else ps_kv.tile([128, 512], F32, tag="kv", name="pk")[:]
                )
                for c in range(CC):
                    nc.tensor.matmul(
                        pk[:, 0:bw],
                        wk[:, c, dc * 128 : (dc + 1) * 128],
                        ctx_t[:, c, lo : lo + bw],
                        start=(c == 0),
                        stop=(c == CC - 1),
                    )
                nc.vector.tensor_scalar_mul(
                    k_t[:, dc, klo : klo + bw], pk[:, 0:bw], FP8_PRE
                )

            def k_fold(part):
                k_t, k_f = (
                    (kT8b[part], kT8fb[part]) if part < 2 else (kT8B, kT8fB)
                )
                dst5 = k_f[:].rearrange("p (hh hl) i m -> p hl hh i m", hl=2)
                for j in range(4):
                    nc.sync.dma_start(
                        out=dst5[:, j // 2, :, j % 2, :],
                        in_=k_t[32 * j : 32 * j + 32, :, :],
                    )

            def v_unit(kt):
                ctx_t, lo = ctx_of(kt * 128)
                pv = ps_kv.tile([128, 512], F32, tag="kv", name="pv")
                for c in range(CC):
                    nc.tensor.matmul(
                        pv[:],
                        ctx_t[:, c, lo : lo + 128],
                        wv[:, c, :],
                        start=(c == 0),
                        stop=(c == CC - 1),
                    )
                v2h = v2t[kt]
                nc.vector.tensor_copy(
                    v2h[:, :, 0:64], pv[:].rearrange("p (h d) -> p h d", d=64)
                )
                nc.vector.tensor_copy(
                    v2h[:, :, 64:65], valid[:, kt : kt + 1].to_broadcast([128, H, 1])
                )

            # PE warmup: dep-free matmuls ramp the p-state while the first
            # DMAs land; results go to a psum bank nobody reads
            # PE warmup: dep-free matmuls ramp the p-state while DMAs land
            dummy = big.tile([128, 512], BF, tag="dummy", name="dummy")
            nc.vector.memset(dummy[:], 0.0)
            wps = ps_kv.tile([128, 512], F32, tag="kv", name="wps")
            for i in range(40):
                nc.tensor.matmul(
                    wps[:], dummy[:, 0:128], dummy[:], start=True, stop=True
                )
            # Q half 0 first (gates rounds 0-7), then K blocks, then Q half 1
            for dc in range(CI):
                q_unit(dc, 0)
            q_fold(0)
            for dc in range(CI):
                k_unit(0, dc, use_slot=True)
            k_fold(0)
            for dc in range(CI):
                k_unit(1, dc, use_slot=True)
            k_fold(1)
            nc.gpsimd.dma_start(
                out=wvo[:], in_=wvo_d[:].rearrange("(o p) f -> p o f", p=128)
            )
            nc.gpsimd.dma_start(
                out=bo_bc[:],
                in_=bass.AP(tensor=bo_d, offset=0, ap=[[0, 128], [1, QUERY_DIM]]),
            )
            nc.sync.dma_start_transpose(ctxTB[:], ctx_d[MA:m_pad, :])

            # production consumed later (Q half 1 at r8; blocks 2+ in pass B),
            # emitted inside pass-A rounds r>=1
            deferred = []
            for dc in range(CI):
                deferred.append(lambda dc=dc: q_unit(dc, 1))
            deferred.append(lambda: q_fold(1))
            for g in range(BA, NB):
                for dc in range(CI):
                    deferred.append(lambda g=g, dc=dc: k_unit(g, dc))
                for kt in range(BLOCKS[g][0], BLOCKS[g][0] + BLOCKS[g][1]):
                    deferred.append(lambda kt=kt: v_unit(kt))
            deferred.append(lambda: k_fold(2))
            di = [0]

            def emit_units(k):
                while k > 0 and di[0] < len(deferred):
                    deferred[di[0]]()
                    di[0] += 1
                    k -= 1

            # ---- attention rounds ----
            def do_pass(qb, h, groups, upsum, interleave, weave=None,
                        dbg_cap=None, two_phase=False):
                last_g = len(groups) - 1

                def scores_exp(gi, kts):
                    gn = len(kts)
                    sc = ps_sc.tile([128, 3, 512], F32, tag="sc", name="sc")
                    for j, kt in enumerate(kts):
                        base = kt * 128
                        if base < 512:
                            k_f, lo = kT8fb[0], base
                        elif base < MA:
                            k_f, lo = kT8fb[1], base - 512
                        else:
                            k_f, lo = kT8fB, base - MA
                        nc.tensor.matmul(
                            sc[:, j, :],
                            k_f[:, h, :, lo : lo + 128],
                            qT8fh[qb][:, h, :, :],
                            start=True,
                            stop=True,
                            perf_mode=PM.DoubleRow,
                        )
                    pt = ptp.tile([128, 3, 512], BF, tag="pt", name="pt")
                    nc.scalar.activation(
                        pt[:, 0:gn, :], sc[:, 0:gn, :], AF.Exp, scale=EXP_SCALE
                    )
                    return pt

                def pv(gi, kts, pt):
                    gn = len(kts)
                    for j, kt in enumerate(kts):
                        for qt in range(4):
                            nc.tensor.matmul(
                                upsum[:, qt, 0:65],
                                pt[:, j, qt * 128 : (qt + 1) * 128],
                                v2t[kt][:, h, :],
                                start=False,
                                stop=(gi == last_g) and (j == gn - 1),
                                skip_group_check=True,
                            )

                if two_phase:
                    pts = [scores_exp(gi, kts) for gi, kts in enumerate(groups)]
                    for gi, kts in enumerate(groups):
                        if weave is not None:
                            with tc.high_priority(offset=-1000000):
                                for fn_ in weave.get(gi, ()):
                                    fn_()
                        pv(gi, kts, pts[gi])
                    return
                for gi, kts in enumerate(groups):
                    pt = scores_exp(gi, kts)
                    if weave is not None:
                        with tc.high_priority(offset=-1000000):
                            for fn_ in weave.get(gi, ()):
                                fn_()
                    if interleave:
                        with tc.high_priority(offset=-1000000):
                            emit_units(2 if di[0] < 13 else 1)
                    pv(gi, kts, pt)

            # pass A: U_a -> SBUF.  Round 0 weaves in K block 1 (+fold) and
            # the A-side V units just ahead of their first consumers.
            a_kts = GR_A
            weave0 = {
                gi: [lambda kt=kt: v_unit(kt) for kt in a_kts[gi]]
                for gi in range(len(a_kts))
            }

            dbg_pt = []
            for r in range(16):
                qb, h = r // 8, r % 8
                ua_ps = ps_u.tile([128, 4, 128], F32, tag="u", name="ua_ps")
                nc.vector.memset(ua_ps[:], 0.0)
                do_pass(
                    qb, h, GR_A, ua_ps, r >= 1,
                    weave=weave0 if r == 0 else None,
                    two_phase=(r == 0),
                )
                nc.vector.tensor_copy(ua[r][:], ua_ps[:, :, 0:65])

            # pass B: U_b + U_a -> normalize -> O; out-proj per query block
            def out_proj_qt(qb, qt):
                oT = otp.tile([128, CI, 128], BF, tag="oT", name="oT")
                nc.sync.dma_start_transpose(oT[:], onat[qb][:, qt, :])
                po = ps_kv.tile([128, 512], F32, tag="kv", name="po")
                for c in range(CI):
                    nc.tensor.matmul(
                        po[:],
                        oT[:, c, :],
                        wo[:, c, :],
                        start=(c == 0),
                        stop=(c == CI - 1),
                    )
                ot = outp.tile([128, QUERY_DIM], F32, tag="ot", name="ot")
                nc.vector.tensor_add(ot[:], po[:], bo_bc[:])
                qrow = (qb * 4 + qt) * 128
                nc.sync.dma_start(out=out_d[qrow : qrow + 128, :], in_=ot[:])

            for r in range(16):
                qb, h = r // 8, r % 8
                ub_ps = ps_u.tile([128, 4, 128], F32, tag="u", name="ub_ps")
                nc.vector.memset(ub_ps[:], 0.0)
                do_pass(qb, h, GR_B, ub_ps, False)
                fin = finp.tile([128, 4, 65], F32, tag="fin", name="fin")
                nc.vector.tensor_add(fin[:], ub_ps[:, :, 0:65], ua[r][:])
                nc.vector.reciprocal(fin[:, :, 64:65], fin[:, :, 64:65])
                for qt in range(4):
                    nc.vector.tensor_scalar_mul(
                        onat[qb][:, qt, h * 64 : (h + 1) * 64],
                        fin[:, qt, 0:64],
                        fin[:, qt, 64:65],
                    )
                if r >= 9 and r % 2 == 1:
                    out_proj_qt(0, (r - 9) // 2)
            # final block: batch transposes, then matmul+add chains, then stores
            oTs = []
            for qt in range(4):
                oT = otp.tile([128, CI, 128], BF, tag="oTf", name=f"oTf{qt}")
                nc.sync.dma_start_transpose(oT[:], onat[1][:, qt, :])
                oTs.append(oT)
            otf = big.tile([128, 4, QUERY_DIM], F32, tag="otf", name="otf")
            for qt in range(4):
                po = ps_kv.tile([128, 512], F32, tag="kv", name="pof")
                for c in range(CI):
                    nc.tensor.matmul(
                        po[:],
                        oTs[qt][:, c, :],
                        wo[:, c, :],
                        start=(c == 0),
                        stop=(c == CI - 1),
                    )
                nc.vector.tensor_add(otf[:, qt, :], po[:], bo_bc[:])
            nc.sync.dma_start(
                out=out_d[512:1024, :].rearrange("(t p) f -> p t f", p=128),
                in_=otf[:],
            )

            if _DBG:
                dumps = [
                    ("d_qT8f0", qT8fh[0], [32, H, 2, 512]),
                    ("d_kT8fb0", kT8fb[0], [32, H, 2, 512]),
                    ("d_kT8b0", kT8b[0], [128, CI, 512]),
                    ("d_v2_0", v2t[0], [128, H, 65]),
                    ("d_ua0", ua[0], [128, 4, 65]),
                    ("d_on0", onat[0], [128, 4, INNER]),
                    ("d_xT0", xTh[0], [128, CQ, 512]),
                    ("d_cT0", ctxT0, [128, CC, 512]),
                ]
                for nm, t, shp in dumps:
                    dt_ = t[:].dtype
                    dd = nc.declare_dram_parameter(nm, shp, dt_, isOutput=True)
                    nc.sync.dma_start(out=dd[:], in_=t[:])

    nc.compile()
    return nc


def kernel(x, context_tensor, mask, Wq, Wk, Wv, Wo, bo):
    from concourse.bass_utils import run_bass_kernel_spmd

    x = np.asarray(x, dtype=np.float32)
    context_tensor = np.asarray(context_tensor, dtype=np.float32)
    mask = np.asarray(mask)
    Wq = np.asarray(Wq, dtype=np.float32)
    Wk = np.asarray(Wk, dtype=np.float32)
    Wv = np.asarray(Wv, dtype=np.float32)
    Wo = np.asarray(Wo, dtype=np.float32)
    bo = np.ascontiguousarray(np.asarray(bo, dtype=np.float32))

    # host-side context compaction using the mask
    meffs = [int(mask[b].sum()) for b in range(B)]
    m_pad = max(M_PAD_MIN, ((max(meffs) + 127) // 128) * 128)
    ctx_c = np.zeros((B, m_pad, CONTEXT_DIM), dtype=BF16)
    val = np.zeros((B, m_pad), dtype=BF16)
    for b in range(B):
        idx = np.flatnonzero(mask[b])
        ctx_c[b, : len(idx)] = context_tensor[b, idx].astype(BF16)
        val[b, : len(idx)] = 1.0

    if m_pad not in _compiled:
        _compiled[m_pad] = _build(m_pad)
    nc = _compiled[m_pad]

    x16 = np.ascontiguousarray(x.astype(BF16))
    wqk16 = np.ascontiguousarray(np.concatenate([Wq, Wk], axis=0).astype(BF16))
    wvo16 = np.ascontiguousarray(np.concatenate([Wv, Wo], axis=0).astype(BF16))

    rows_per_core = N // (NCORES // B)  # 1024
    in_maps = []
    for d in range(NCORES):
        b = d // (NCORES // B)
        r0 = (d % (NCORES // B)) * rows_per_core
        in_maps.append(
            {
                "xs": np.ascontiguousarray(x16[b, r0 : r0 + rows_per_core]),
                "ctx": ctx_c[b],
                "valid": val[b],
                "Wqk": wqk16,
                "Wvo": wvo16,
                "bo": bo,
            }
        )

    res = run_bass_kernel_spmd(nc, in_maps, list(range(NCORES)))
    global _last_results
    _last_results = res
    out = np.empty((B, N, QUERY_DIM), dtype=np.float32)
    for d in range(NCORES):
        b = d // (NCORES // B)
        r0 = (d % (NCORES // B)) * rows_per_core
        out[b, r0 : r0 + rows_per_core] = res.results[d]["out"]
    return out


# revision 39
# speedup vs baseline: 1.2640x; 1.0068x over previous
"""Cross-attention Bass kernel for 8 trn2 NeuronCores.

Sharding: core d handles batch b = d//4 and query rows [(d%4)*1024, (d%4+1)*1024)
of that batch, computing all 8 heads (no collectives). The context is compacted
on the host using the mask (masked rows dropped, zero-padded to a multiple of
128), which preserves softmax semantics exactly.

Design (v3):
- All data bf16 except: PSUM (f32), scores path (fp8e4 DoubleRow).
- x^T / ctx^T produced by XBAR DMA-transpose straight out of DRAM (inputs are
  host-cast to bf16); weight tiles use the matching (o p) f -> p o f layout.
- Scores: Q^T/K^T drained to fp8 (x16) and partition-folded via small
  SBUF->SBUF DMAs into [32, (h, 2, m)] so one DoubleRow matmul per (head,
  k-tile) contracts all 64 head dims at 0.5 cycles/row.
- P = exp(S) on ACT straight out of 3-bank PSUM score groups into bf16; the
  softmax scale and fp8 prescale fold into the exp. ACT runs ONLY exps.
- PV uses P^T chunks as stationary and V (+ones column for the denominator)
  as the 65-wide moving operand. U accumulates per (qtile,head) in one PSUM
  bank (4 slots, first-write start marks the zero region).
- Two passes over k-groups (A: first 8 k-tiles, B: rest) bound U-psum
  residency, so pass-A rounds start after two context blocks; blocks 2+ are
  produced inside pass-A rounds to keep PE busy under the ACT exp stream.
- Normalization is a per-partition broadcast multiply on DVE (q is partition
  dim after PV); O is DMA-transposed for the output projection.
- Every tile is written-once-read-later at whole-tile granularity (per-block
  / per-half tiles) to avoid false WAR/RAW serialization in Tile's tracker.
"""
import numpy as np
import ml_dtypes

B, N, M = 2, 4096, 4096
QUERY_DIM, CONTEXT_DIM = 512, 768
H, D = 8, 64
INNER = H * D  # 512
NCORES = 8
N_DEV = (B * N) // NCORES  # 1024 query rows per core
M_PAD_MIN = 1792

BF16 = ml_dtypes.bfloat16
SCALE = float(D) ** -0.5
FP8_PRE = 16.0  # fp8 prescale on Q and K; exp scale divides by 16*16

_compiled = {}
_DBG = False


def _build(m_pad):
    from concourse import bacc
    import concourse.bass as bass
    import concourse.mybir as mybir
    import concourse.tile as tile

    F32 = mybir.dt.float32
    BF = mybir.dt.bfloat16
    F8 = mybir.dt.float8e4
    AF = mybir.ActivationFunctionType
    PM = mybir.MatmulPerfMode

    KT = m_pad // 128  # k-tiles (17 for 2176)
    # context blocks of up to 4 k-tiles
    BLOCKS = [(4 * g, min(4, KT - 4 * g)) for g in range((KT + 3) // 4)]
    NB = len(BLOCKS)
    BA = 2  # pass A covers blocks [0, BA), pass B the rest
    CQ = QUERY_DIM // 128  # 4
    CC = CONTEXT_DIM // 128  # 6
    CI = INNER // 128  # 4
    EXP_SCALE = SCALE / (FP8_PRE * FP8_PRE)

    # exp groups of up to 3 k-tiles within each pass (psum: 3 banks x 2 bufs)
    def make_groups(kts):
        out = []
        i = 0
        while i < len(kts):
            out.append(kts[i : i + 3])
            i += 3
        return out

    A_KT = [kt for g0, gn in BLOCKS[:BA] for kt in range(g0, g0 + gn)]
    B_KT = [kt for g0, gn in BLOCKS[BA:] for kt in range(g0, g0 + gn)]
    GR_A = make_groups(A_KT)  # e.g. [[0,1,2],[3,4,5],[6,7]]
    GR_B = make_groups(B_KT)  # e.g. [[8,9,10],[11,12,13],[14,15,16]]

    nc = bacc.Bacc()
    xs_d = nc.declare_dram_parameter("xs", [N_DEV, QUERY_DIM], BF, isOutput=False)
    ctx_d = nc.declare_dram_parameter("ctx", [m_pad, CONTEXT_DIM], BF, isOutput=False)
    val_d = nc.declare_dram_parameter("valid", [m_pad], BF, isOutput=False)
    wqk_d = nc.declare_dram_parameter(
        "Wqk", [QUERY_DIM + CONTEXT_DIM, INNER], BF, isOutput=False
    )
    wvo_d = nc.declare_dram_parameter(
        "Wvo", [CONTEXT_DIM + INNER, INNER], BF, isOutput=False
    )
    bo_d = nc.declare_dram_parameter("bo", [QUERY_DIM], F32, isOutput=False)
    out_d = nc.declare_dram_parameter("out", [N_DEV, QUERY_DIM], F32, isOutput=True)

    with tile.TileContext(nc) as tc:
        with (
            tc.tile_pool(name="big", bufs=1) as big,
            tc.tile_pool(name="pt", bufs=3) as ptp,
            tc.tile_pool(name="fin", bufs=2) as finp,
            tc.tile_pool(name="otp", bufs=2) as otp,
            tc.tile_pool(name="outp", bufs=2) as outp,
            tc.tile_pool(name="ps_sc", bufs=2, space="PSUM") as ps_sc,
            tc.tile_pool(name="ps_u", bufs=1, space="PSUM") as ps_u,
            tc.tile_pool(name="ps_kv", bufs=1, space="PSUM") as ps_kv,
        ):
            # ---- static SBUF tiles (each written once, at one spot) ----
            wqk = big.tile([128, CQ + CC, INNER], BF, tag="wqk", name="wqk")
            wvo = big.tile([128, CC + CI, INNER], BF, tag="wvo", name="wvo")
            wq = wqk[:, 0:CQ, :]
            wk = wqk[:, CQ : CQ + CC, :]
            wv = wvo[:, 0:CC, :]
            wo = wvo[:, CC : CC + CI, :]
            bo_bc = big.tile([128, QUERY_DIM], F32, tag="bo", name="bo")
            valid = big.tile([128, KT], BF, tag="valid", name="valid")
            xTh = [
                big.tile([128, CQ, 512], BF, tag=f"xT{i}", name=f"xT{i}")
                for i in range(2)
            ]
            MA = min(1024, m_pad)  # pass-A context columns
            MB = m_pad - MA
            ctxT0 = big.tile([128, CC, 512], BF, tag="cT0", name="cT0")
            ctxT1 = big.tile([128, CC, 512], BF, tag="cT1", name="cT1")
            ctxTB = big.tile([128, CC, MB], BF, tag="cTB", name="cTB")
            qT8h = [
                big.tile([128, CI, 512], F8, tag=f"qT8_{i}", name=f"qT8_{i}")
                for i in range(2)
            ]
            qT8fh = [
                big.tile([32, H, 2, 512], F8, tag=f"qT8f_{i}", name=f"qT8f_{i}")
                for i in range(2)
            ]
            kT8b = [
                big.tile([128, CI, 512], F8, tag=f"kT8b{i}", name=f"kT8b{i}")
                for i in range(2)
            ]
            kT8B = big.tile([128, CI, MB], F8, tag="kT8B", name="kT8B")
            kT8fb = [
                big.tile([32, H, 2, 512], F8, tag=f"kfb{i}", name=f"kfb{i}")
                for i in range(2)
            ]
            kT8fB = big.tile([32, H, 2, MB], F8, tag="kfB", name="kfB")
            v2t = [
                big.tile([128, H, 65], BF, tag=f"v2_{t}", name=f"v2_{t}")
                for t in range(KT)
            ]
            ua = [
                big.tile([128, 4, 65], F32, tag=f"ua{r}", name=f"ua{r}")
                for r in range(16)
            ]
            onat = [
                big.tile([128, 4, INNER], BF, tag=f"on{qb}", name=f"on{qb}")
                for qb in range(2)
            ]

            # ---- loads + input transposes (DMA device is serialized: Q path
            # first, then ctx blocks in consumption order) ----
            # Few, large DMAs: chains between unlike DMAs then cost little.
            nc.sync.dma_start_transpose(xTh[0][:], xs_d[0:512, :])
            nc.sync.dma_start_transpose(xTh[1][:], xs_d[512:1024, :])
            nc.sync.dma_start_transpose(ctxT0[:], ctx_d[0:512, :])
            nc.scalar.dma_start(
                out=wqk[:], in_=wqk_d[:].rearrange("(o p) f -> p o f", p=128)
            )
            nc.scalar.dma_start(
                out=valid[:], in_=val_d[:].rearrange("(t p) -> p t", p=128)
            )
            nc.sync.dma_start_transpose(ctxT1[:], ctx_d[512:1024, :])

            # rotating psum slots for the prologue projection chains: the
            # two 3-bank sc tiles provide 6 independent banks
            _slot_state = {"tile": None, "i": 0}

            def next_slot():
                if _slot_state["i"] % 3 == 0:
                    _slot_state["tile"] = ps_sc.tile(
                        [128, 3, 512], F32, tag="sc", name="slot"
                    )
                j = _slot_state["i"] % 3
                _slot_state["i"] += 1
                return _slot_state["tile"][:, j, :]

            # ---- Q projection unit: one (dc, qf) chain -> fp8 drain ----
            def q_unit(dc, qf):
                pq = next_slot()
                for c in range(CQ):
                    nc.tensor.matmul(
                        pq,
                        wq[:, c, dc * 128 : (dc + 1) * 128],
                        xTh[qf][:, c, :],
                        start=(c == 0),
                        stop=(c == CQ - 1),
                    )
                nc.vector.tensor_scalar_mul(qT8h[qf][:, dc, :], pq, FP8_PRE)

            def q_fold(qf):
                # one DMA per 32-partition quarter: all 4 dc chunks at once
                # (dst heads h = 2*dc + j//2 are stride-2 in h)
                dst5 = qT8fh[qf][:].rearrange(
                    "p (hh hl) i m -> p hl hh i m", hl=2
                )
                for j in range(4):
                    nc.sync.dma_start(
                        out=dst5[:, j // 2, :, j % 2, :],
                        in_=qT8h[qf][32 * j : 32 * j + 32, :, :],
                    )

            # ---- K/V production units (blocks 0,1 separate; B merged) ----
            def ctx_of(base):
                if base < 512:
                    return ctxT0, base
                if base < MA:
                    return ctxT1, base - 512
                return ctxTB, base - MA

            def k_unit(g, dc, use_slot=False):
                g0, gn = BLOCKS[g]
                bw = gn * 128
                ctx_t, lo = ctx_of(g0 * 128)
                k_t, klo = (
                    (kT8b[g], 0) if g < 2 else (kT8B, g0 * 128 - MA)
                )
                pk = (
                    next_slot()
                    if use_slot
                    else ps_kv.tile([128, 512], F32, tag="kv", name="pk")[:]
                )
                for c in range(CC):
                    nc.tensor.matmul(
                        pk[:, 0:bw],
                        wk[:, c, dc * 128 : (dc + 1) * 128],
                        ctx_t[:, c, lo : lo + bw],
                        start=(c == 0),
                        stop=(c == CC - 1),
                    )
                nc.vector.tensor_scalar_mul(
                    k_t[:, dc, klo : klo + bw], pk[:, 0:bw], FP8_PRE
                )

            def k_fold(part):
                k_t, k_f = (
                    (kT8b[part], kT8fb[part]) if part < 2 else (kT8B, kT8fB)
                )
                dst5 = k_f[:].rearrange("p (hh hl) i m -> p hl hh i m", hl=2)
                for j in range(4):
                    nc.sync.dma_start(
                        out=dst5[:, j // 2, :, j % 2, :],
                        in_=k_t[32 * j : 32 * j + 32, :, :],
                    )

            def v_unit(kt):
                ctx_t, lo = ctx_of(kt * 128)
                pv = ps_kv.tile([128, 512], F32, tag="kv", name="pv")
                for c in range(CC):
                    nc.tensor.matmul(
                        pv[:],
                        ctx_t[:, c, lo : lo + 128],
                        wv[:, c, :],
                        start=(c == 0),
                        stop=(c == CC - 1),
                    )
                v2h = v2t[kt]
                nc.vector.tensor_copy(
                    v2h[:, :, 0:64], pv[:].rearrange("p (h d) -> p h d", d=64)
                )
                nc.vector.tensor_copy(
                    v2h[:, :, 64:65], valid[:, kt : kt + 1].to_broadcast([128, H, 1])
                )

            # PE warmup: dep-free matmuls ramp the p-state while the first
            # DMAs land; results go to a psum bank nobody reads
            # PE warmup: dep-free matmuls ramp the p-state while DMAs land
            dummy = big.tile([128, 512], BF, tag="dummy", name="dummy")
            nc.vector.memset(dummy[:], 0.0)
            wps = ps_kv.tile([128, 512], F32, tag="kv", name="wps")
            for i in range(40):
                nc.tensor.matmul(
                    wps[:], dummy[:, 0:128], dummy[:], start=True, stop=True
                )
            # Q half 0 first (gates rounds 0-7), then K blocks, then Q half 1
            for dc in range(CI):
                q_unit(dc, 0)
            q_fold(0)
            for dc in range(CI):
                k_unit(0, dc, use_slot=True)
            k_fold(0)
            for dc in range(CI):
                k_unit(1, dc, use_slot=True)
            k_fold(1)
            nc.gpsimd.dma_start(
                out=wvo[:], in_=wvo_d[:].rearrange("(o p) f -> p o f", p=128)
            )
            nc.gpsimd.dma_start(
                out=bo_bc[:],
                in_=bass.AP(tensor=bo_d, offset=0, ap=[[0, 128], [1, QUERY_DIM]]),
            )
            nc.sync.dma_start_transpose(ctxTB[:], ctx_d[MA:m_pad, :])

            # production consumed later (Q half 1 at r8; blocks 2+ in pass B),
            # emitted inside pass-A rounds r>=1
            deferred = []
            for dc in range(CI):
                deferred.append(lambda dc=dc: q_unit(dc, 1))
            deferred.append(lambda: q_fold(1))
            for g in range(BA, NB):
                for dc in range(CI):
                    deferred.append(lambda g=g, dc=dc: k_unit(g, dc))
                for kt in range(BLOCKS[g][0], BLOCKS[g][0] + BLOCKS[g][1]):
                    deferred.append(lambda kt=kt: v_unit(kt))
            deferred.append(lambda: k_fold(2))
            di = [0]

            def emit_units(k):
                while k > 0 and di[0] < len(deferred):
                    deferred[di[0]]()
                    di[0] += 1
                    k -= 1

            # ---- attention rounds ----
            def do_pass(qb, h, groups, upsum, interleave, weave=None,
                        dbg_cap=None, two_phase=False):
                last_g = len(groups) - 1

                def scores_exp(gi, kts):
                    gn = len(kts)
                    sc = ps_sc.tile([128, 3, 512], F32, tag="sc", name="sc")
                    for j, kt in enumerate(kts):
                        base = kt * 128
                        if base < 512:
                            k_f, lo = kT8fb[0], base
                        elif base < MA:
                            k_f, lo = kT8fb[1], base - 512
                        else:
                            k_f, lo = kT8fB, base - MA
                        nc.tensor.matmul(
                            sc[:, j, :],
                            k_f[:, h, :, lo : lo + 128],
                            qT8fh[qb][:, h, :, :],
                            start=True,
                            stop=True,
                            perf_mode=PM.DoubleRow,
                        )
                    pt = ptp.tile([128, 3, 512], BF, tag="pt", name="pt")
                    nc.scalar.activation(
                        pt[:, 0:gn, :], sc[:, 0:gn, :], AF.Exp, scale=EXP_SCALE
                    )
                    return pt

                def pv(gi, kts, pt):
                    gn = len(kts)
                    for j, kt in enumerate(kts):
                        for qt in range(4):
                            nc.tensor.matmul(
                                upsum[:, qt, 0:65],
                                pt[:, j, qt * 128 : (qt + 1) * 128],
                                v2t[kt][:, h, :],
                                start=False,
                                stop=(gi == last_g) and (j == gn - 1),
                                skip_group_check=True,
                            )

                if two_phase:
                    pts = [scores_exp(gi, kts) for gi, kts in enumerate(groups)]
                    for gi, kts in enumerate(groups):
                        if weave is not None:
                            with tc.high_priority(offset=-1000000):
                                for fn_ in weave.get(gi, ()):
                                    fn_()
                        pv(gi, kts, pts[gi])
                    return
                for gi, kts in enumerate(groups):
                    pt = scores_exp(gi, kts)
                    if weave is not None:
                        with tc.high_priority(offset=-1000000):
                            for fn_ in weave.get(gi, ()):
                                fn_()
                    if interleave:
                        with tc.high_priority(offset=-1000000):
                            emit_units(2 if di[0] < 13 else 1)
                    pv(gi, kts, pt)

            # pass A: U_a -> SBUF.  Round 0 weaves in K block 1 (+fold) and
            # the A-side V units just ahead of their first consumers.
            a_kts = GR_A
            weave0 = {
                gi: [lambda kt=kt: v_unit(kt) for kt in a_kts[gi]]
                for gi in range(len(a_kts))
            }

            dbg_pt = []
            for r in range(16):
                qb, h = r // 8, r % 8
                ua_ps = ps_u.tile([128, 4, 128], F32, tag="u", name="ua_ps")
                nc.vector.memset(ua_ps[:], 0.0)
                do_pass(
                    qb, h, GR_A, ua_ps, r >= 1,
                    weave=weave0 if r == 0 else None,
                    two_phase=(r == 0),
                )
                nc.vector.tensor_copy(ua[r][:], ua_ps[:, :, 0:65])

            # pass B: U_b + U_a -> normalize -> O; out-proj per query block
            def out_proj_qt(qb, qt):
                oT = otp.tile([128, CI, 128], BF, tag="oT", name="oT")
                nc.sync.dma_start_transpose(oT[:], onat[qb][:, qt, :])
                po = ps_kv.tile([128, 512], F32, tag="kv", name="po")
                for c in range(CI):
                    nc.tensor.matmul(
                        po[:],
                        oT[:, c, :],
                        wo[:, c, :],
                        start=(c == 0),
                        stop=(c == CI - 1),
                    )
                ot = outp.tile([128, QUERY_DIM], F32, tag="ot", name="ot")
                nc.vector.tensor_add(ot[:], po[:], bo_bc[:])
                qrow = (qb * 4 + qt) * 128
                nc.sync.dma_start(out=out_d[qrow : qrow + 128, :], in_=ot[:])

            for r in range(16):
                qb, h = r // 8, r % 8
                ub_ps = ps_u.tile([128, 4, 128], F32, tag="u", name="ub_ps")
                nc.vector.memset(ub_ps[:], 0.0)
                do_pass(qb, h, GR_B, ub_ps, False)
                fin = finp.tile([128, 4, 65], F32, tag="fin", name="fin")
                if r == 15:
                    # last round: per-qtile chains so the output projection
                    # starts as soon as each qtile's normalize lands
                    for qt in range(4):
                        nc.vector.tensor_add(
                            fin[:, qt, :], ub_ps[:, qt, 0:65], ua[r][:, qt, :]
                        )
                        nc.vector.reciprocal(
                            fin[:, qt, 64:65], fin[:, qt, 64:65]
                        )
                        nc.vector.tensor_scalar_mul(
                            onat[qb][:, qt, h * 64 : (h + 1) * 64],
                            fin[:, qt, 0:64],
                            fin[:, qt, 64:65],
                        )
                else:
                    nc.vector.tensor_add(fin[:], ub_ps[:, :, 0:65], ua[r][:])
                    nc.vector.reciprocal(fin[:, :, 64:65], fin[:, :, 64:65])
                    for qt in range(4):
                        nc.vector.tensor_scalar_mul(
                            onat[qb][:, qt, h * 64 : (h + 1) * 64],
                            fin[:, qt, 0:64],
                            fin[:, qt, 64:65],
                        )
                if r >= 9 and r % 2 == 1:
                    out_proj_qt(0, (r - 9) // 2)
            # final block: batch transposes, then matmul+add chains, then stores
            otf = big.tile([128, 4, QUERY_DIM], F32, tag="otf", name="otf")
            for qt in range(4):
                oT = otp.tile([128, CI, 128], BF, tag="oTf", name=f"oTf{qt}")
                nc.sync.dma_start_transpose(oT[:], onat[1][:, qt, :])
                po = ps_sc.tile([128, 3, 512], F32, tag="sc", name="pof")
                for c in range(CI):
                    nc.tensor.matmul(
                        po[:, 0, :],
                        oT[:, c, :],
                        wo[:, c, :],
                        start=(c == 0),
                        stop=(c == CI - 1),
                    )
                nc.vector.tensor_add(otf[:, qt, :], po[:, 0, :], bo_bc[:])
            nc.sync.dma_start(
                out=out_d[512:1024, :].rearrange("(t p) f -> p t f", p=128),
                in_=otf[:],
            )

            if _DBG:
                dumps = [
                    ("d_qT8f0", qT8fh[0], [32, H, 2, 512]),
                    ("d_kT8fb0", kT8fb[0], [32, H, 2, 512]),
                    ("d_kT8b0", kT8b[0], [128, CI, 512]),
                    ("d_v2_0", v2t[0], [128, H, 65]),
                    ("d_ua0", ua[0], [128, 4, 65]),
                    ("d_on0", onat[0], [128, 4, INNER]),
                    ("d_xT0", xTh[0], [128, CQ, 512]),
                    ("d_cT0", ctxT0, [128, CC, 512]),
                ]
                for nm, t, shp in dumps:
                    dt_ = t[:].dtype
                    dd = nc.declare_dram_parameter(nm, shp, dt_, isOutput=True)
                    nc.sync.dma_start(out=dd[:], in_=t[:])

    nc.compile()
    return nc


def kernel(x, context_tensor, mask, Wq, Wk, Wv, Wo, bo):
    from concourse.bass_utils import run_bass_kernel_spmd

    x = np.asarray(x, dtype=np.float32)
    context_tensor = np.asarray(context_tensor, dtype=np.float32)
    mask = np.asarray(mask)
    Wq = np.asarray(Wq, dtype=np.float32)
    Wk = np.asarray(Wk, dtype=np.float32)
    Wv = np.asarray(Wv, dtype=np.float32)
    Wo = np.asarray(Wo, dtype=np.float32)
    bo = np.ascontiguousarray(np.asarray(bo, dtype=np.float32))

    # host-side context compaction using the mask
    meffs = [int(mask[b].sum()) for b in range(B)]
    m_pad = max(M_PAD_MIN, ((max(meffs) + 127) // 128) * 128)
    ctx_c = np.zeros((B, m_pad, CONTEXT_DIM), dtype=BF16)
    val = np.zeros((B, m_pad), dtype=BF16)
    for b in range(B):
        idx = np.flatnonzero(mask[b])
        ctx_c[b, : len(idx)] = context_tensor[b, idx].astype(BF16)
        val[b, : len(idx)] = 1.0

    if m_pad not in _compiled:
        _compiled[m_pad] = _build(m_pad)
    nc = _compiled[m_pad]

    x16 = np.ascontiguousarray(x.astype(BF16))
    wqk16 = np.ascontiguousarray(np.concatenate([Wq, Wk], axis=0).astype(BF16))
    wvo16 = np.ascontiguousarray(np.concatenate([Wv, Wo], axis=0).astype(BF16))

    rows_per_core = N // (NCORES // B)  # 1024
    in_maps = []
    for d in range(NCORES):
        b = d // (NCORES // B)
        r0 = (d % (NCORES // B)) * rows_per_core
        in_maps.append(
            {
                "xs": np.ascontiguousarray(x16[b, r0 : r0 + rows_per_core]),
                "ctx": ctx_c[b],
                "valid": val[b],
                "Wqk": wqk16,
                "Wvo": wvo16,
                "bo": bo,
            }
        )

    res = run_bass_kernel_spmd(nc, in_maps, list(range(NCORES)))
    global _last_results
    _last_results = res
    out = np.empty((B, N, QUERY_DIM), dtype=np.float32)
    for d in range(NCORES):
        b = d // (NCORES // B)
        r0 = (d % (NCORES // B)) * rows_per_core
        out[b, r0 : r0 + rows_per_core] = res.results[d]["out"]
    return out


# revision 44
# speedup vs baseline: 1.2660x; 1.0016x over previous
"""Cross-attention Bass kernel for 8 trn2 NeuronCores.

Sharding: core d handles batch b = d//4 and query rows [(d%4)*1024, (d%4+1)*1024)
of that batch, computing all 8 heads (no collectives). The context is compacted
on the host using the mask (masked rows dropped, zero-padded to a multiple of
128), which preserves softmax semantics exactly.

Design (v3):
- All data bf16 except: PSUM (f32), scores path (fp8e4 DoubleRow).
- x^T / ctx^T produced by XBAR DMA-transpose straight out of DRAM (inputs are
  host-cast to bf16); weight tiles use the matching (o p) f -> p o f layout.
- Scores: Q^T/K^T drained to fp8 (x16) and partition-folded via small
  SBUF->SBUF DMAs into [32, (h, 2, m)] so one DoubleRow matmul per (head,
  k-tile) contracts all 64 head dims at 0.5 cycles/row.
- P = exp(S) on ACT straight out of 3-bank PSUM score groups into bf16; the
  softmax scale and fp8 prescale fold into the exp. ACT runs ONLY exps.
- PV uses P^T chunks as stationary and V (+ones column for the denominator)
  as the 65-wide moving operand. U accumulates per (qtile,head) in one PSUM
  bank (4 slots, first-write start marks the zero region).
- Two passes over k-groups (A: first 8 k-tiles, B: rest) bound U-psum
  residency, so pass-A rounds start after two context blocks; blocks 2+ are
  produced inside pass-A rounds to keep PE busy under the ACT exp stream.
- Normalization is a per-partition broadcast multiply on DVE (q is partition
  dim after PV); O is DMA-transposed for the output projection.
- Every tile is written-once-read-later at whole-tile granularity (per-block
  / per-half tiles) to avoid false WAR/RAW serialization in Tile's tracker.
"""
import numpy as np
import ml_dtypes

B, N, M = 2, 4096, 4096
QUERY_DIM, CONTEXT_DIM = 512, 768
H, D = 8, 64
INNER = H * D  # 512
NCORES = 8
N_DEV = (B * N) // NCORES  # 1024 query rows per core
M_PAD_MIN = 1792

BF16 = ml_dtypes.bfloat16
SCALE = float(D) ** -0.5
FP8_PRE = 16.0  # fp8 prescale on Q and K; exp scale divides by 16*16

_compiled = {}
_DBG = False


def _build(m_pad):
    from concourse import bacc
    import concourse.bass as bass
    import concourse.mybir as mybir
    import concourse.tile as tile

    F32 = mybir.dt.float32
    BF = mybir.dt.bfloat16
    F8 = mybir.dt.float8e4
    AF = mybir.ActivationFunctionType
    PM = mybir.MatmulPerfMode

    KT = m_pad // 128  # k-tiles (17 for 2176)
    # context blocks of up to 4 k-tiles
    BLOCKS = [(4 * g, min(4, KT - 4 * g)) for g in range((KT + 3) // 4)]
    NB = len(BLOCKS)
    BA = 2  # pass A covers blocks [0, BA), pass B the rest
    CQ = QUERY_DIM // 128  # 4
    CC = CONTEXT_DIM // 128  # 6
    CI = INNER // 128  # 4
    EXP_SCALE = SCALE / (FP8_PRE * FP8_PRE)

    # exp groups of up to 3 k-tiles within each pass (psum: 3 banks x 2 bufs)
    def make_groups(kts):
        out = []
        i = 0
        while i < len(kts):
            out.append(kts[i : i + 3])
            i += 3
        return out

    A_KT = [kt for g0, gn in BLOCKS[:BA] for kt in range(g0, g0 + gn)]
    B_KT = [kt for g0, gn in BLOCKS[BA:] for kt in range(g0, g0 + gn)]
    GR_A = make_groups(A_KT)  # e.g. [[0,1,2],[3,4,5],[6,7]]
    GR_B = make_groups(B_KT)  # e.g. [[8,9,10],[11,12,13],[14,15,16]]

    nc = bacc.Bacc()
    xs_d = nc.declare_dram_parameter("xs", [N_DEV, QUERY_DIM], BF, isOutput=False)
    ctx_d = nc.declare_dram_parameter("ctx", [m_pad, CONTEXT_DIM], BF, isOutput=False)
    val_d = nc.declare_dram_parameter("valid", [m_pad], BF, isOutput=False)
    wqk_d = nc.declare_dram_parameter(
        "Wqk", [QUERY_DIM + CONTEXT_DIM, INNER], BF, isOutput=False
    )
    wvo_d = nc.declare_dram_parameter(
        "Wvo", [CONTEXT_DIM + INNER, INNER], BF, isOutput=False
    )
    bo_d = nc.declare_dram_parameter("bo", [QUERY_DIM], F32, isOutput=False)
    out_d = nc.declare_dram_parameter("out", [N_DEV, QUERY_DIM], F32, isOutput=True)

    with tile.TileContext(nc) as tc:
        with (
            tc.tile_pool(name="big", bufs=1) as big,
            tc.tile_pool(name="pt", bufs=3) as ptp,
            tc.tile_pool(name="fin", bufs=2) as finp,
            tc.tile_pool(name="otp", bufs=2) as otp,
            tc.tile_pool(name="outp", bufs=2) as outp,
            tc.tile_pool(name="ps_sc", bufs=2, space="PSUM") as ps_sc,
            tc.tile_pool(name="ps_u", bufs=1, space="PSUM") as ps_u,
            tc.tile_pool(name="ps_kv", bufs=1, space="PSUM") as ps_kv,
        ):
            # ---- static SBUF tiles (each written once, at one spot) ----
            wqk = big.tile([128, CQ + CC, INNER], BF, tag="wqk", name="wqk")
            wvo = big.tile([128, CC + CI, INNER], BF, tag="wvo", name="wvo")
            wq = wqk[:, 0:CQ, :]
            wk = wqk[:, CQ : CQ + CC, :]
            wv = wvo[:, 0:CC, :]
            wo = wvo[:, CC : CC + CI, :]
            bo_bc = big.tile([128, QUERY_DIM], F32, tag="bo", name="bo")
            valid = big.tile([128, KT], BF, tag="valid", name="valid")
            xTh = [
                big.tile([128, CQ, 512], BF, tag=f"xT{i}", name=f"xT{i}")
                for i in range(2)
            ]
            MA = min(1024, m_pad)  # pass-A context columns
            MB = m_pad - MA
            ctxT0 = big.tile([128, CC, 512], BF, tag="cT0", name="cT0")
            ctxT1 = big.tile([128, CC, 512], BF, tag="cT1", name="cT1")
            ctxTB = big.tile([128, CC, MB], BF, tag="cTB", name="cTB")
            qT8h = [
                big.tile([128, CI, 512], F8, tag=f"qT8_{i}", name=f"qT8_{i}")
                for i in range(2)
            ]
            qT8fh = [
                big.tile([32, H, 2, 512], F8, tag=f"qT8f_{i}", name=f"qT8f_{i}")
                for i in range(2)
            ]
            kT8b = [
                big.tile([128, CI, 512], F8, tag=f"kT8b{i}", name=f"kT8b{i}")
                for i in range(2)
            ]
            kT8B = big.tile([128, CI, MB], F8, tag="kT8B", name="kT8B")
            kT8fb = [
                big.tile([32, H, 2, 512], F8, tag=f"kfb{i}", name=f"kfb{i}")
                for i in range(2)
            ]
            kT8fB = big.tile([32, H, 2, MB], F8, tag="kfB", name="kfB")
            v2t = [
                big.tile([128, H, 65], BF, tag=f"v2_{t}", name=f"v2_{t}")
                for t in range(KT)
            ]
            ua = [
                big.tile([128, 4, 65], F32, tag=f"ua{r}", name=f"ua{r}")
                for r in range(16)
            ]
            onat = [
                big.tile([128, 4, INNER], BF, tag=f"on{qb}", name=f"on{qb}")
                for qb in range(2)
            ]

            # ---- loads + input transposes (DMA device is serialized: Q path
            # first, then ctx blocks in consumption order) ----
            # Few, large DMAs: chains between unlike DMAs then cost little.
            nc.sync.dma_start_transpose(xTh[0][:], xs_d[0:512, :])
            nc.sync.dma_start_transpose(xTh[1][:], xs_d[512:1024, :])
            nc.sync.dma_start_transpose(ctxT0[:], ctx_d[0:512, :])
            nc.scalar.dma_start(
                out=wqk[:], in_=wqk_d[:].rearrange("(o p) f -> p o f", p=128)
            )
            nc.scalar.dma_start(
                out=valid[:], in_=val_d[:].rearrange("(t p) -> p t", p=128)
            )
            nc.sync.dma_start_transpose(ctxT1[:], ctx_d[512:1024, :])

            # rotating psum slots for the prologue projection chains: the
            # two 3-bank sc tiles provide 6 independent banks
            _slot_state = {"tile": None, "i": 0}

            def next_slot():
                if _slot_state["i"] % 3 == 0:
                    _slot_state["tile"] = ps_sc.tile(
                        [128, 3, 512], F32, tag="sc", name="slot"
                    )
                j = _slot_state["i"] % 3
                _slot_state["i"] += 1
                return _slot_state["tile"][:, j, :]

            # ---- Q projection unit: one (dc, qf) chain -> fp8 drain ----
            def q_unit(dc, qf):
                pq = next_slot()
                for c in range(CQ):
                    nc.tensor.matmul(
                        pq,
                        wq[:, c, dc * 128 : (dc + 1) * 128],
                        xTh[qf][:, c, :],
                        start=(c == 0),
                        stop=(c == CQ - 1),
                    )
                nc.vector.tensor_scalar_mul(qT8h[qf][:, dc, :], pq, FP8_PRE)

            def q_fold(qf):
                # one DMA per 32-partition quarter: all 4 dc chunks at once
                # (dst heads h = 2*dc + j//2 are stride-2 in h)
                dst5 = qT8fh[qf][:].rearrange(
                    "p (hh hl) i m -> p hl hh i m", hl=2
                )
                for j in range(4):
                    nc.sync.dma_start(
                        out=dst5[:, j // 2, :, j % 2, :],
                        in_=qT8h[qf][32 * j : 32 * j + 32, :, :],
                    )

            # ---- K/V production units (blocks 0,1 separate; B merged) ----
            def ctx_of(base):
                if base < 512:
                    return ctxT0, base
                if base < MA:
                    return ctxT1, base - 512
                return ctxTB, base - MA

            def k_unit(g, dc, use_slot=False):
                g0, gn = BLOCKS[g]
                bw = gn * 128
                ctx_t, lo = ctx_of(g0 * 128)
                k_t, klo = (
                    (kT8b[g], 0) if g < 2 else (kT8B, g0 * 128 - MA)
                )
                pk = (
                    next_slot()
                    if use_slot
                    else ps_kv.tile([128, 512], F32, tag="kv", name="pk")[:]
                )
                for c in range(CC):
                    nc.tensor.matmul(
                        pk[:, 0:bw],
                        wk[:, c, dc * 128 : (dc + 1) * 128],
                        ctx_t[:, c, lo : lo + bw],
                        start=(c == 0),
                        stop=(c == CC - 1),
                    )
                nc.vector.tensor_scalar_mul(
                    k_t[:, dc, klo : klo + bw], pk[:, 0:bw], FP8_PRE
                )

            def k_fold(part):
                k_t, k_f = (
                    (kT8b[part], kT8fb[part]) if part < 2 else (kT8B, kT8fB)
                )
                dst5 = k_f[:].rearrange("p (hh hl) i m -> p hl hh i m", hl=2)
                for j in range(4):
                    nc.sync.dma_start(
                        out=dst5[:, j // 2, :, j % 2, :],
                        in_=k_t[32 * j : 32 * j + 32, :, :],
                    )

            def v_unit(kt):
                ctx_t, lo = ctx_of(kt * 128)
                pv = ps_kv.tile([128, 512], F32, tag="kv", name="pv")
                for c in range(CC):
                    nc.tensor.matmul(
                        pv[:],
                        ctx_t[:, c, lo : lo + 128],
                        wv[:, c, :],
                        start=(c == 0),
                        stop=(c == CC - 1),
                    )
                v2h = v2t[kt]
                nc.vector.tensor_copy(
                    v2h[:, :, 0:64], pv[:].rearrange("p (h d) -> p h d", d=64)
                )
                nc.vector.tensor_copy(
                    v2h[:, :, 64:65], valid[:, kt : kt + 1].to_broadcast([128, H, 1])
                )

            # PE warmup: dep-free matmuls ramp the p-state while the first
            # DMAs land; results go to a psum bank nobody reads
            # PE warmup: dep-free matmuls ramp the p-state while DMAs land
            dummy = big.tile([128, 512], BF, tag="dummy", name="dummy")
            nc.vector.memset(dummy[:], 0.0)
            wps = ps_kv.tile([128, 512], F32, tag="kv", name="wps")
            for i in range(24):
                nc.tensor.matmul(
                    wps[:], dummy[:, 0:128], dummy[:], start=True, stop=True
                )
            # Q half 0 first (gates rounds 0-7), then K blocks, then Q half 1
            for dc in range(CI):
                q_unit(dc, 0)
            q_fold(0)
            for dc in range(CI):
                k_unit(0, dc, use_slot=True)
            k_fold(0)
            for dc in range(CI):
                k_unit(1, dc, use_slot=True)
            k_fold(1)
            nc.gpsimd.dma_start(
                out=wvo[:], in_=wvo_d[:].rearrange("(o p) f -> p o f", p=128)
            )
            nc.gpsimd.dma_start(
                out=bo_bc[:],
                in_=bass.AP(tensor=bo_d, offset=0, ap=[[0, 128], [1, QUERY_DIM]]),
            )
            nc.sync.dma_start_transpose(ctxTB[:], ctx_d[MA:m_pad, :])

            # production consumed later (Q half 1 at r8; blocks 2+ in pass B),
            # emitted inside pass-A rounds r>=1
            deferred = []
            for dc in range(CI):
                deferred.append(lambda dc=dc: q_unit(dc, 1))
            deferred.append(lambda: q_fold(1))
            for g in range(BA, NB):
                for dc in range(CI):
                    deferred.append(lambda g=g, dc=dc: k_unit(g, dc))
                for kt in range(BLOCKS[g][0], BLOCKS[g][0] + BLOCKS[g][1]):
                    deferred.append(lambda kt=kt: v_unit(kt))
            deferred.append(lambda: k_fold(2))
            di = [0]

            def emit_units(k):
                while k > 0 and di[0] < len(deferred):
                    deferred[di[0]]()
                    di[0] += 1
                    k -= 1

            # ---- attention rounds ----
            def do_pass(qb, h, groups, upsum, interleave, weave=None,
                        dbg_cap=None, two_phase=False):
                last_g = len(groups) - 1

                def scores_exp(gi, kts):
                    gn = len(kts)
                    sc = ps_sc.tile([128, 3, 512], F32, tag="sc", name="sc")
                    for j, kt in enumerate(kts):
                        base = kt * 128
                        if base < 512:
                            k_f, lo = kT8fb[0], base
                        elif base < MA:
                            k_f, lo = kT8fb[1], base - 512
                        else:
                            k_f, lo = kT8fB, base - MA
                        nc.tensor.matmul(
                            sc[:, j, :],
                            k_f[:, h, :, lo : lo + 128],
                            qT8fh[qb][:, h, :, :],
                            start=True,
                            stop=True,
                            perf_mode=PM.DoubleRow,
                        )
                    pt = ptp.tile([128, 3, 512], BF, tag="pt", name="pt")
                    nc.scalar.activation(
                        pt[:, 0:gn, :], sc[:, 0:gn, :], AF.Exp, scale=EXP_SCALE
                    )
                    return pt

                def pv(gi, kts, pt):
                    gn = len(kts)
                    for j, kt in enumerate(kts):
                        for qt in range(4):
                            nc.tensor.matmul(
                                upsum[:, qt, 0:65],
                                pt[:, j, qt * 128 : (qt + 1) * 128],
                                v2t[kt][:, h, :],
                                start=False,
                                stop=(gi == last_g) and (j == gn - 1),
                                skip_group_check=True,
                            )

                if two_phase:
                    pts = [scores_exp(gi, kts) for gi, kts in enumerate(groups)]
                    for gi, kts in enumerate(groups):
                        if weave is not None:
                            with tc.high_priority(offset=-1000000):
                                for fn_ in weave.get(gi, ()):
                                    fn_()
                        pv(gi, kts, pts[gi])
                    return
                for gi, kts in enumerate(groups):
                    pt = scores_exp(gi, kts)
                    if weave is not None:
                        with tc.high_priority(offset=-1000000):
                            for fn_ in weave.get(gi, ()):
                                fn_()
                    if interleave:
                        with tc.high_priority(offset=-1000000):
                            emit_units(2 if di[0] < 13 else 1)
                    pv(gi, kts, pt)

            # pass A: U_a -> SBUF.  Round 0 weaves in K block 1 (+fold) and
            # the A-side V units just ahead of their first consumers.
            a_kts = GR_A
            weave0 = {
                gi: [lambda kt=kt: v_unit(kt) for kt in a_kts[gi]]
                for gi in range(len(a_kts))
            }

            dbg_pt = []
            for r in range(16):
                qb, h = r // 8, r % 8
                ua_ps = ps_u.tile([128, 4, 128], F32, tag="u", name="ua_ps")
                nc.vector.memset(ua_ps[:], 0.0)
                do_pass(
                    qb, h, GR_A, ua_ps, r >= 1,
                    weave=weave0 if r == 0 else None,
                    two_phase=(r == 0),
                )
                nc.vector.tensor_copy(ua[r][:], ua_ps[:, :, 0:65])

            # pass B: U_b + U_a -> normalize -> O; out-proj per query block
            def out_proj_qt(qb, qt):
                oT = otp.tile([128, CI, 128], BF, tag="oT", name="oT")
                nc.sync.dma_start_transpose(oT[:], onat[qb][:, qt, :])
                po = ps_kv.tile([128, 512], F32, tag="kv", name="po")
                for c in range(CI):
                    nc.tensor.matmul(
                        po[:],
                        oT[:, c, :],
                        wo[:, c, :],
                        start=(c == 0),
                        stop=(c == CI - 1),
                    )
                ot = outp.tile([128, QUERY_DIM], F32, tag="ot", name="ot")
                nc.vector.tensor_add(ot[:], po[:], bo_bc[:])
                qrow = (qb * 4 + qt) * 128
                nc.sync.dma_start(out=out_d[qrow : qrow + 128, :], in_=ot[:])

            for r in range(16):
                qb, h = r // 8, r % 8
                ub_ps = ps_u.tile([128, 4, 128], F32, tag="u", name="ub_ps")
                nc.vector.memset(ub_ps[:], 0.0)
                do_pass(qb, h, GR_B, ub_ps, False)
                fin = finp.tile([128, 4, 65], F32, tag="fin", name="fin")
                if r == 15:
                    # last round: per-qtile chains so the output projection
                    # starts as soon as each qtile's normalize lands
                    for qt in range(4):
                        nc.vector.tensor_add(
                            fin[:, qt, :], ub_ps[:, qt, 0:65], ua[r][:, qt, :]
                        )
                        nc.vector.reciprocal(
                            fin[:, qt, 64:65], fin[:, qt, 64:65]
                        )
                        nc.vector.tensor_scalar_mul(
                            onat[qb][:, qt, h * 64 : (h + 1) * 64],
                            fin[:, qt, 0:64],
                            fin[:, qt, 64:65],
                        )
                else:
                    nc.vector.tensor_add(fin[:], ub_ps[:, :, 0:65], ua[r][:])
                    nc.vector.reciprocal(fin[:, :, 64:65], fin[:, :, 64:65])
                    for qt in range(4):
                        nc.vector.tensor_scalar_mul(
                            onat[qb][:, qt, h * 64 : (h + 1) * 64],
                            fin[:, qt, 0:64],
                            fin[:, qt, 64:65],
                        )
                if r >= 8 and r % 2 == 0:
                    out_proj_qt(0, (r - 8) // 2)
            # final block: batch transposes, then matmul+add chains, then stores
            otf = big.tile([128, 4, QUERY_DIM], F32, tag="otf", name="otf")
            for qt in range(4):
                oT = otp.tile([128, CI, 128], BF, tag="oTf", name=f"oTf{qt}")
                nc.sync.dma_start_transpose(oT[:], onat[1][:, qt, :])
                po = ps_sc.tile([128, 3, 512], F32, tag="sc", name="pof")
                for c in range(CI):
                    nc.tensor.matmul(
                        po[:, 0, :],
                        oT[:, c, :],
                        wo[:, c, :],
                        start=(c == 0),
                        stop=(c == CI - 1),
                    )
                nc.vector.tensor_add(otf[:, qt, :], po[:, 0, :], bo_bc[:])
            nc.sync.dma_start(
                out=out_d[512:1024, :].rearrange("(t p) f -> p t f", p=128),
                in_=otf[:],
            )

            if _DBG:
                dumps = [
                    ("d_qT8f0", qT8fh[0], [32, H, 2, 512]),
                    ("d_kT8fb0", kT8fb[0], [32, H, 2, 512]),
                    ("d_kT8b0", kT8b[0], [128, CI, 512]),
                    ("d_v2_0", v2t[0], [128, H, 65]),
                    ("d_ua0", ua[0], [128, 4, 65]),
                    ("d_on0", onat[0], [128, 4, INNER]),
                    ("d_xT0", xTh[0], [128, CQ, 512]),
                    ("d_cT0", ctxT0, [128, CC, 512]),
                ]
                for nm, t, shp in dumps:
                    dt_ = t[:].dtype
                    dd = nc.declare_dram_parameter(nm, shp, dt_, isOutput=True)
                    nc.sync.dma_start(out=dd[:], in_=t[:])

    nc.compile()
    return nc


def kernel(x, context_tensor, mask, Wq, Wk, Wv, Wo, bo):
    from concourse.bass_utils import run_bass_kernel_spmd

    x = np.asarray(x, dtype=np.float32)
    context_tensor = np.asarray(context_tensor, dtype=np.float32)
    mask = np.asarray(mask)
    Wq = np.asarray(Wq, dtype=np.float32)
    Wk = np.asarray(Wk, dtype=np.float32)
    Wv = np.asarray(Wv, dtype=np.float32)
    Wo = np.asarray(Wo, dtype=np.float32)
    bo = np.ascontiguousarray(np.asarray(bo, dtype=np.float32))

    # host-side context compaction using the mask
    meffs = [int(mask[b].sum()) for b in range(B)]
    m_pad = max(M_PAD_MIN, ((max(meffs) + 127) // 128) * 128)
    ctx_c = np.zeros((B, m_pad, CONTEXT_DIM), dtype=BF16)
    val = np.zeros((B, m_pad), dtype=BF16)
    for b in range(B):
        idx = np.flatnonzero(mask[b])
        ctx_c[b, : len(idx)] = context_tensor[b, idx].astype(BF16)
        val[b, : len(idx)] = 1.0

    if m_pad not in _compiled:
        _compiled[m_pad] = _build(m_pad)
    nc = _compiled[m_pad]

    x16 = np.ascontiguousarray(x.astype(BF16))
    wqk16 = np.ascontiguousarray(np.concatenate([Wq, Wk], axis=0).astype(BF16))
    wvo16 = np.ascontiguousarray(np.concatenate([Wv, Wo], axis=0).astype(BF16))

    rows_per_core = N // (NCORES // B)  # 1024
    in_maps = []
    for d in range(NCORES):
        b = d // (NCORES // B)
        r0 = (d % (NCORES // B)) * rows_per_core
        in_maps.append(
            {
                "xs": np.ascontiguousarray(x16[b, r0 : r0 + rows_per_core]),
                "ctx": ctx_c[b],
                "valid": val[b],
                "Wqk": wqk16,
                "Wvo": wvo16,
                "bo": bo,
            }
        )

    res = run_bass_kernel_spmd(nc, in_maps, list(range(NCORES)))
    global _last_results
    _last_results = res
    out = np.empty((B, N, QUERY_DIM), dtype=np.float32)
    for d in range(NCORES):
        b = d // (NCORES // B)
        r0 = (d % (NCORES // B)) * rows_per_core
        out[b, r0 : r0 + rows_per_core] = res.results[d]["out"]
    return out


# revision 47
# speedup vs baseline: 1.2789x; 1.0102x over previous
"""Cross-attention Bass kernel for 8 trn2 NeuronCores.

Sharding: core d handles batch b = d//4 and query rows [(d%4)*1024, (d%4+1)*1024)
of that batch, computing all 8 heads (no collectives). The context is compacted
on the host using the mask (masked rows dropped, zero-padded to a multiple of
128), which preserves softmax semantics exactly.

Design (v3):
- All data bf16 except: PSUM (f32), scores path (fp8e4 DoubleRow).
- x^T / ctx^T produced by XBAR DMA-transpose straight out of DRAM (inputs are
  host-cast to bf16); weight tiles use the matching (o p) f -> p o f layout.
- Scores: Q^T/K^T drained to fp8 (x16) and partition-folded via small
  SBUF->SBUF DMAs into [32, (h, 2, m)] so one DoubleRow matmul per (head,
  k-tile) contracts all 64 head dims at 0.5 cycles/row.
- P = exp(S) on ACT straight out of 3-bank PSUM score groups into bf16; the
  softmax scale and fp8 prescale fold into the exp. ACT runs ONLY exps.
- PV uses P^T chunks as stationary and V (+ones column for the denominator)
  as the 65-wide moving operand. U accumulates per (qtile,head) in one PSUM
  bank (4 slots, first-write start marks the zero region).
- Two passes over k-groups (A: first 8 k-tiles, B: rest) bound U-psum
  residency, so pass-A rounds start after two context blocks; blocks 2+ are
  produced inside pass-A rounds to keep PE busy under the ACT exp stream.
- Normalization is a per-partition broadcast multiply on DVE (q is partition
  dim after PV); O is DMA-transposed for the output projection.
- Every tile is written-once-read-later at whole-tile granularity (per-block
  / per-half tiles) to avoid false WAR/RAW serialization in Tile's tracker.
"""
import numpy as np
import ml_dtypes

B, N, M = 2, 4096, 4096
QUERY_DIM, CONTEXT_DIM = 512, 768
H, D = 8, 64
INNER = H * D  # 512
NCORES = 8
N_DEV = (B * N) // NCORES  # 1024 query rows per core
M_PAD_MIN = 1792

BF16 = ml_dtypes.bfloat16
SCALE = float(D) ** -0.5
FP8_PRE = 16.0  # fp8 prescale on Q and K; exp scale divides by 16*16

_compiled = {}
_DBG = False


def _build(m_pad):
    from concourse import bacc
    import concourse.bass as bass
    import concourse.mybir as mybir
    import concourse.tile as tile

    F32 = mybir.dt.float32
    BF = mybir.dt.bfloat16
    F8 = mybir.dt.float8e4
    AF = mybir.ActivationFunctionType
    PM = mybir.MatmulPerfMode

    KT = m_pad // 128  # k-tiles (17 for 2176)
    # context blocks of up to 4 k-tiles
    BLOCKS = [(4 * g, min(4, KT - 4 * g)) for g in range((KT + 3) // 4)]
    NB = len(BLOCKS)
    BA = 2  # pass A covers blocks [0, BA), pass B the rest
    CQ = QUERY_DIM // 128  # 4
    CC = CONTEXT_DIM // 128  # 6
    CI = INNER // 128  # 4
    EXP_SCALE = SCALE / (FP8_PRE * FP8_PRE)

    # exp groups of up to 3 k-tiles within each pass (psum: 3 banks x 2 bufs)
    def make_groups(kts):
        out = []
        i = 0
        while i < len(kts):
            out.append(kts[i : i + 3])
            i += 3
        return out

    A_KT = [kt for g0, gn in BLOCKS[:BA] for kt in range(g0, g0 + gn)]
    B_KT = [kt for g0, gn in BLOCKS[BA:] for kt in range(g0, g0 + gn)]
    GR_A = make_groups(A_KT)  # e.g. [[0,1,2],[3,4,5],[6,7]]
    GR_B = make_groups(B_KT)  # e.g. [[8,9,10],[11,12,13],[14,15,16]]

    nc = bacc.Bacc()
    xs_d = nc.declare_dram_parameter("xs", [N_DEV, QUERY_DIM], BF, isOutput=False)
    ctx_d = nc.declare_dram_parameter("ctx", [m_pad, CONTEXT_DIM], BF, isOutput=False)
    val_d = nc.declare_dram_parameter("valid", [m_pad], BF, isOutput=False)
    wqk_d = nc.declare_dram_parameter(
        "Wqk", [QUERY_DIM + CONTEXT_DIM, INNER], BF, isOutput=False
    )
    wvo_d = nc.declare_dram_parameter(
        "Wvo", [CONTEXT_DIM + INNER, INNER], BF, isOutput=False
    )
    bo_d = nc.declare_dram_parameter("bo", [QUERY_DIM], F32, isOutput=False)
    out_d = nc.declare_dram_parameter("out", [N_DEV, QUERY_DIM], F32, isOutput=True)

    with tile.TileContext(nc) as tc:
        with (
            tc.tile_pool(name="big", bufs=1) as big,
            tc.tile_pool(name="pt", bufs=3) as ptp,
            tc.tile_pool(name="fin", bufs=2) as finp,
            tc.tile_pool(name="otp", bufs=2) as otp,
            tc.tile_pool(name="outp", bufs=2) as outp,
            tc.tile_pool(name="ps_sc", bufs=2, space="PSUM") as ps_sc,
            tc.tile_pool(name="ps_u", bufs=1, space="PSUM") as ps_u,
            tc.tile_pool(name="ps_kv", bufs=1, space="PSUM") as ps_kv,
        ):
            # ---- static SBUF tiles (each written once, at one spot) ----
            wqk = big.tile([128, CQ + CC, INNER], BF, tag="wqk", name="wqk")
            wvo = big.tile([128, CC + CI, INNER], BF, tag="wvo", name="wvo")
            wq = wqk[:, 0:CQ, :]
            wk = wqk[:, CQ : CQ + CC, :]
            wv = wvo[:, 0:CC, :]
            wo = wvo[:, CC : CC + CI, :]
            bo_bc = big.tile([128, QUERY_DIM], F32, tag="bo", name="bo")
            valid = big.tile([128, KT], BF, tag="valid", name="valid")
            xTh = [
                big.tile([128, CQ, 512], BF, tag=f"xT{i}", name=f"xT{i}")
                for i in range(2)
            ]
            MA = min(1024, m_pad)  # pass-A context columns
            MB = m_pad - MA
            ctxT0 = big.tile([128, CC, 512], BF, tag="cT0", name="cT0")
            ctxT1 = big.tile([128, CC, 512], BF, tag="cT1", name="cT1")
            ctxTB = big.tile([128, CC, MB], BF, tag="cTB", name="cTB")
            qT8h = [
                big.tile([128, CI, 512], F8, tag=f"qT8_{i}", name=f"qT8_{i}")
                for i in range(2)
            ]
            qT8fh = [
                big.tile([32, H, 2, 512], F8, tag=f"qT8f_{i}", name=f"qT8f_{i}")
                for i in range(2)
            ]
            kT8b = [
                big.tile([128, CI, 512], F8, tag=f"kT8b{i}", name=f"kT8b{i}")
                for i in range(2)
            ]
            kT8B = big.tile([128, CI, MB], F8, tag="kT8B", name="kT8B")
            kT8fb = [
                big.tile([32, H, 2, 512], F8, tag=f"kfb{i}", name=f"kfb{i}")
                for i in range(2)
            ]
            kT8fB = big.tile([32, H, 2, MB], F8, tag="kfB", name="kfB")
            v2t = [
                big.tile([128, H, 65], BF, tag=f"v2_{t}", name=f"v2_{t}")
                for t in range(KT)
            ]
            ua = [
                big.tile([128, 4, 65], F32, tag=f"ua{r}", name=f"ua{r}")
                for r in range(16)
            ]
            onat = [
                big.tile([128, 4, INNER], BF, tag=f"on{qb}", name=f"on{qb}")
                for qb in range(2)
            ]

            # ---- loads + input transposes (DMA device is serialized: Q path
            # first, then ctx blocks in consumption order) ----
            # Few, large DMAs: chains between unlike DMAs then cost little.
            nc.sync.dma_start_transpose(xTh[0][:], xs_d[0:512, :])
            nc.sync.dma_start_transpose(xTh[1][:], xs_d[512:1024, :])
            nc.sync.dma_start_transpose(ctxT0[:], ctx_d[0:512, :])
            nc.scalar.dma_start(
                out=wqk[:], in_=wqk_d[:].rearrange("(o p) f -> p o f", p=128)
            )
            nc.scalar.dma_start(
                out=valid[:], in_=val_d[:].rearrange("(t p) -> p t", p=128)
            )
            nc.sync.dma_start_transpose(ctxT1[:], ctx_d[512:1024, :])

            # rotating psum slots for the prologue projection chains: the
            # two 3-bank sc tiles provide 6 independent banks
            _slot_state = {"tile": None, "i": 0}

            def next_slot():
                if _slot_state["i"] % 3 == 0:
                    _slot_state["tile"] = ps_sc.tile(
                        [128, 3, 512], F32, tag="sc", name="slot"
                    )
                j = _slot_state["i"] % 3
                _slot_state["i"] += 1
                return _slot_state["tile"][:, j, :]

            # ---- Q projection unit: one (dc, qf) chain -> fp8 drain ----
            def q_unit(dc, qf):
                pq = next_slot()
                for c in range(CQ):
                    nc.tensor.matmul(
                        pq,
                        wq[:, c, dc * 128 : (dc + 1) * 128],
                        xTh[qf][:, c, :],
                        start=(c == 0),
                        stop=(c == CQ - 1),
                    )
                nc.vector.tensor_scalar_mul(qT8h[qf][:, dc, :], pq, FP8_PRE)

            def q_fold(qf):
                # one DMA per 32-partition quarter: all 4 dc chunks at once
                # (dst heads h = 2*dc + j//2 are stride-2 in h)
                dst5 = qT8fh[qf][:].rearrange(
                    "p (hh hl) i m -> p hl hh i m", hl=2
                )
                for j in range(4):
                    nc.sync.dma_start(
                        out=dst5[:, j // 2, :, j % 2, :],
                        in_=qT8h[qf][32 * j : 32 * j + 32, :, :],
                    )

            # ---- K/V production units (blocks 0,1 separate; B merged) ----
            def ctx_of(base):
                if base < 512:
                    return ctxT0, base
                if base < MA:
                    return ctxT1, base - 512
                return ctxTB, base - MA

            def k_unit(g, dc, use_slot=False):
                g0, gn = BLOCKS[g]
                bw = gn * 128
                ctx_t, lo = ctx_of(g0 * 128)
                k_t, klo = (
                    (kT8b[g], 0) if g < 2 else (kT8B, g0 * 128 - MA)
                )
                pk = (
                    next_slot()
                    if use_slot
                    else ps_kv.tile([128, 512], F32, tag="kv", name="pk")[:]
                )
                for c in range(CC):
                    nc.tensor.matmul(
                        pk[:, 0:bw],
                        wk[:, c, dc * 128 : (dc + 1) * 128],
                        ctx_t[:, c, lo : lo + bw],
                        start=(c == 0),
                        stop=(c == CC - 1),
                    )
                nc.vector.tensor_scalar_mul(
                    k_t[:, dc, klo : klo + bw], pk[:, 0:bw], FP8_PRE
                )

            def k_fold(part):
                k_t, k_f = (
                    (kT8b[part], kT8fb[part]) if part < 2 else (kT8B, kT8fB)
                )
                dst5 = k_f[:].rearrange("p (hh hl) i m -> p hl hh i m", hl=2)
                for j in range(4):
                    nc.sync.dma_start(
                        out=dst5[:, j // 2, :, j % 2, :],
                        in_=k_t[32 * j : 32 * j + 32, :, :],
                    )

            def v_unit(kt):
                ctx_t, lo = ctx_of(kt * 128)
                pv = ps_kv.tile([128, 512], F32, tag="kv", name="pv")
                for c in range(CC):
                    nc.tensor.matmul(
                        pv[:],
                        ctx_t[:, c, lo : lo + 128],
                        wv[:, c, :],
                        start=(c == 0),
                        stop=(c == CC - 1),
                    )
                v2h = v2t[kt]
                nc.vector.tensor_copy(
                    v2h[:, :, 0:64], pv[:].rearrange("p (h d) -> p h d", d=64)
                )
                nc.vector.tensor_copy(
                    v2h[:, :, 64:65], valid[:, kt : kt + 1].to_broadcast([128, H, 1])
                )

            # PE warmup: dep-free matmuls ramp the p-state while the first
            # DMAs land; results go to a psum bank nobody reads
            # PE warmup: dep-free matmuls ramp the p-state while DMAs land
            dummy = big.tile([128, 512], BF, tag="dummy", name="dummy")
            nc.vector.memset(dummy[:], 0.0)
            wps = ps_kv.tile([128, 512], F32, tag="kv", name="wps")
            for i in range(24):
                nc.tensor.matmul(
                    wps[:], dummy[:, 0:128], dummy[:], start=True, stop=True
                )
            # Q half 0 first (gates rounds 0-7), then K blocks, then Q half 1
            for dc in range(CI):
                q_unit(dc, 0)
            q_fold(0)
            for dc in range(CI):
                k_unit(0, dc, use_slot=True)
            k_fold(0)
            for dc in range(CI):
                k_unit(1, dc, use_slot=True)
            k_fold(1)
            nc.gpsimd.dma_start(
                out=wvo[:], in_=wvo_d[:].rearrange("(o p) f -> p o f", p=128)
            )
            nc.gpsimd.dma_start(
                out=bo_bc[:],
                in_=bass.AP(tensor=bo_d, offset=0, ap=[[0, 128], [1, QUERY_DIM]]),
            )
            nc.sync.dma_start_transpose(ctxTB[:], ctx_d[MA:m_pad, :])

            # production consumed later (Q half 1 at r8; blocks 2+ in pass B),
            # emitted inside pass-A rounds r>=1
            deferred = []
            for dc in range(CI):
                deferred.append(lambda dc=dc: q_unit(dc, 1))
            deferred.append(lambda: q_fold(1))
            for g in range(BA, NB):
                for dc in range(CI):
                    deferred.append(lambda g=g, dc=dc: k_unit(g, dc))
                for kt in range(BLOCKS[g][0], BLOCKS[g][0] + BLOCKS[g][1]):
                    deferred.append(lambda kt=kt: v_unit(kt))
            deferred.append(lambda: k_fold(2))
            di = [0]

            def emit_units(k):
                while k > 0 and di[0] < len(deferred):
                    deferred[di[0]]()
                    di[0] += 1
                    k -= 1

            # ---- attention rounds ----
            def do_pass(qb, h, groups, upsum, interleave, weave=None,
                        dbg_cap=None, two_phase=False):
                last_g = len(groups) - 1

                def scores_exp(gi, kts):
                    gn = len(kts)
                    sc = ps_sc.tile([128, 3, 512], F32, tag="sc", name="sc")
                    for j, kt in enumerate(kts):
                        base = kt * 128
                        if base < 512:
                            k_f, lo = kT8fb[0], base
                        elif base < MA:
                            k_f, lo = kT8fb[1], base - 512
                        else:
                            k_f, lo = kT8fB, base - MA
                        nc.tensor.matmul(
                            sc[:, j, :],
                            k_f[:, h, :, lo : lo + 128],
                            qT8fh[qb][:, h, :, :],
                            start=True,
                            stop=True,
                            perf_mode=PM.DoubleRow,
                        )
                    pt = ptp.tile([128, 3, 512], BF, tag="pt", name="pt")
                    nc.scalar.activation(
                        pt[:, 0:gn, :], sc[:, 0:gn, :], AF.Exp, scale=EXP_SCALE
                    )
                    return pt

                def pv(gi, kts, pt):
                    gn = len(kts)
                    for j, kt in enumerate(kts):
                        for qt in range(4):
                            nc.tensor.matmul(
                                upsum[:, qt, 0:65],
                                pt[:, j, qt * 128 : (qt + 1) * 128],
                                v2t[kt][:, h, :],
                                start=False,
                                stop=(gi == last_g) and (j == gn - 1),
                                skip_group_check=True,
                            )

                if two_phase:
                    pts = [scores_exp(gi, kts) for gi, kts in enumerate(groups)]
                    for gi, kts in enumerate(groups):
                        if weave is not None:
                            with tc.high_priority(offset=-1000000):
                                for fn_ in weave.get(gi, ()):
                                    fn_()
                        pv(gi, kts, pts[gi])
                    return
                for gi, kts in enumerate(groups):
                    pt = scores_exp(gi, kts)
                    if weave is not None:
                        with tc.high_priority(offset=-1000000):
                            for fn_ in weave.get(gi, ()):
                                fn_()
                    if interleave:
                        with tc.high_priority(offset=-1000000):
                            emit_units(2 if di[0] < 13 else 1)
                    pv(gi, kts, pt)

            # pass A: U_a -> SBUF.  Round 0 weaves in K block 1 (+fold) and
            # the A-side V units just ahead of their first consumers.
            a_kts = GR_A
            weave0 = {
                gi: [lambda kt=kt: v_unit(kt) for kt in a_kts[gi]]
                for gi in range(len(a_kts))
            }

            dbg_pt = []
            for r in range(16):
                qb, h = r // 8, r % 8
                ua_ps = ps_u.tile([128, 4, 128], F32, tag="u", name="ua_ps")
                nc.vector.memset(ua_ps[:], 0.0)
                do_pass(
                    qb, h, GR_A, ua_ps, r >= 1,
                    weave=weave0 if r == 0 else None,
                    two_phase=(r == 0),
                )
                nc.vector.tensor_copy(ua[r][:], ua_ps[:, :, 0:65])

            # pass B: U_b + U_a -> normalize -> O; out-proj per query block
            def out_proj_qt(qb, qt):
                oT = otp.tile([128, CI, 128], BF, tag="oT", name="oT")
                nc.sync.dma_start_transpose(oT[:], onat[qb][:, qt, :])
                po = ps_kv.tile([128, 512], F32, tag="kv", name="po")
                for c in range(CI):
                    nc.tensor.matmul(
                        po[:],
                        oT[:, c, :],
                        wo[:, c, :],
                        start=(c == 0),
                        stop=(c == CI - 1),
                    )
                ot = outp.tile([128, QUERY_DIM], F32, tag="ot", name="ot")
                nc.vector.tensor_add(ot[:], po[:], bo_bc[:])
                qrow = (qb * 4 + qt) * 128
                nc.sync.dma_start(out=out_d[qrow : qrow + 128, :], in_=ot[:])

            for r in range(16):
                qb, h = r // 8, r % 8
                if r % 2 == 1:
                    ub_ps = ps_kv.tile([128, 4, 128], F32, tag="kv", name="ub_ps")
                else:
                    ub_ps = ps_u.tile([128, 4, 128], F32, tag="u", name="ub_ps")
                nc.vector.memset(ub_ps[:], 0.0)
                do_pass(qb, h, GR_B, ub_ps, False)
                fin = finp.tile([128, 4, 65], F32, tag="fin", name="fin")
                if r == 15:
                    # last round: per-qtile chains so the output projection
                    # starts as soon as each qtile's normalize lands
                    for qt in range(4):
                        nc.vector.tensor_add(
                            fin[:, qt, :], ub_ps[:, qt, 0:65], ua[r][:, qt, :]
                        )
                        nc.vector.reciprocal(
                            fin[:, qt, 64:65], fin[:, qt, 64:65]
                        )
                        nc.vector.tensor_scalar_mul(
                            onat[qb][:, qt, h * 64 : (h + 1) * 64],
                            fin[:, qt, 0:64],
                            fin[:, qt, 64:65],
                        )
                else:
                    nc.vector.tensor_add(fin[:], ub_ps[:, :, 0:65], ua[r][:])
                    nc.vector.reciprocal(fin[:, :, 64:65], fin[:, :, 64:65])
                    for qt in range(4):
                        nc.vector.tensor_scalar_mul(
                            onat[qb][:, qt, h * 64 : (h + 1) * 64],
                            fin[:, qt, 0:64],
                            fin[:, qt, 64:65],
                        )
                if r >= 8 and r % 2 == 0:
                    out_proj_qt(0, (r - 8) // 2)
            # final block: batch transposes, then matmul+add chains, then stores
            otf = big.tile([128, 4, QUERY_DIM], F32, tag="otf", name="otf")
            for qt in range(4):
                oT = otp.tile([128, CI, 128], BF, tag="oTf", name=f"oTf{qt}")
                nc.sync.dma_start_transpose(oT[:], onat[1][:, qt, :])
                po = ps_sc.tile([128, 3, 512], F32, tag="sc", name="pof")
                for c in range(CI):
                    nc.tensor.matmul(
                        po[:, 0, :],
                        oT[:, c, :],
                        wo[:, c, :],
                        start=(c == 0),
                        stop=(c == CI - 1),
                    )
                nc.vector.tensor_add(otf[:, qt, :], po[:, 0, :], bo_bc[:])
            nc.sync.dma_start(
                out=out_d[512:1024, :].rearrange("(t p) f -> p t f", p=128),
                in_=otf[:],
            )

            if _DBG:
                dumps = [
                    ("d_qT8f0", qT8fh[0], [32, H, 2, 512]),
                    ("d_kT8fb0", kT8fb[0], [32, H, 2, 512]),
                    ("d_kT8b0", kT8b[0], [128, CI, 512]),
                    ("d_v2_0", v2t[0], [128, H, 65]),
                    ("d_ua0", ua[0], [128, 4, 65]),
                    ("d_on0", onat[0], [128, 4, INNER]),
                    ("d_xT0", xTh[0], [128, CQ, 512]),
                    ("d_cT0", ctxT0, [128, CC, 512]),
                ]
                for nm, t, shp in dumps:
                    dt_ = t[:].dtype
                    dd = nc.declare_dram_parameter(nm, shp, dt_, isOutput=True)
                    nc.sync.dma_start(out=dd[:], in_=t[:])

    nc.compile()
    return nc


def kernel(x, context_tensor, mask, Wq, Wk, Wv, Wo, bo):
    from concourse.bass_utils import run_bass_kernel_spmd

    x = np.asarray(x, dtype=np.float32)
    context_tensor = np.asarray(context_tensor, dtype=np.float32)
    mask = np.asarray(mask)
    Wq = np.asarray(Wq, dtype=np.float32)
    Wk = np.asarray(Wk, dtype=np.float32)
    Wv = np.asarray(Wv, dtype=np.float32)
    Wo = np.asarray(Wo, dtype=np.float32)
    bo = np.ascontiguousarray(np.asarray(bo, dtype=np.float32))

    # host-side context compaction using the mask
    meffs = [int(mask[b].sum()) for b in range(B)]
    m_pad = max(M_PAD_MIN, ((max(meffs) + 127) // 128) * 128)
    ctx_c = np.zeros((B, m_pad, CONTEXT_DIM), dtype=BF16)
    val = np.zeros((B, m_pad), dtype=BF16)
    for b in range(B):
        idx = np.flatnonzero(mask[b])
        ctx_c[b, : len(idx)] = context_tensor[b, idx].astype(BF16)
        val[b, : len(idx)] = 1.0

    if m_pad not in _compiled:
        _compiled[m_pad] = _build(m_pad)
    nc = _compiled[m_pad]

    x16 = np.ascontiguousarray(x.astype(BF16))
    wqk16 = np.ascontiguousarray(np.concatenate([Wq, Wk], axis=0).astype(BF16))
    wvo16 = np.ascontiguousarray(np.concatenate([Wv, Wo], axis=0).astype(BF16))

    rows_per_core = N // (NCORES // B)  # 1024
    in_maps = []
    for d in range(NCORES):
        b = d // (NCORES // B)
        r0 = (d % (NCORES // B)) * rows_per_core
        in_maps.append(
            {
                "xs": np.ascontiguousarray(x16[b, r0 : r0 + rows_per_core]),
                "ctx": ctx_c[b],
                "valid": val[b],
                "Wqk": wqk16,
                "Wvo": wvo16,
                "bo": bo,
            }
        )

    res = run_bass_kernel_spmd(nc, in_maps, list(range(NCORES)))
    global _last_results
    _last_results = res
    out = np.empty((B, N, QUERY_DIM), dtype=np.float32)
    for d in range(NCORES):
        b = d // (NCORES // B)
        r0 = (d % (NCORES // B)) * rows_per_core
        out[b, r0 : r0 + rows_per_core] = res.results[d]["out"]
    return out


# revision 50
# speedup vs baseline: 1.2801x; 1.0009x over previous
"""Cross-attention Bass kernel for 8 trn2 NeuronCores.

Sharding: core d handles batch b = d//4 and query rows [(d%4)*1024, (d%4+1)*1024)
of that batch, computing all 8 heads (no collectives). The context is compacted
on the host using the mask (masked rows dropped, zero-padded to a multiple of
128), which preserves softmax semantics exactly.

Design (v3):
- All data bf16 except: PSUM (f32), scores path (fp8e4 DoubleRow).
- x^T / ctx^T produced by XBAR DMA-transpose straight out of DRAM (inputs are
  host-cast to bf16); weight tiles use the matching (o p) f -> p o f layout.
- Scores: Q^T/K^T drained to fp8 (x16) and partition-folded via small
  SBUF->SBUF DMAs into [32, (h, 2, m)] so one DoubleRow matmul per (head,
  k-tile) contracts all 64 head dims at 0.5 cycles/row.
- P = exp(S) on ACT straight out of 3-bank PSUM score groups into bf16; the
  softmax scale and fp8 prescale fold into the exp. ACT runs ONLY exps.
- PV uses P^T chunks as stationary and V (+ones column for the denominator)
  as the 65-wide moving operand. U accumulates per (qtile,head) in one PSUM
  bank (4 slots, first-write start marks the zero region).
- Two passes over k-groups (A: first 8 k-tiles, B: rest) bound U-psum
  residency, so pass-A rounds start after two context blocks; blocks 2+ are
  produced inside pass-A rounds to keep PE busy under the ACT exp stream.
- Normalization is a per-partition broadcast multiply on DVE (q is partition
  dim after PV); O is DMA-transposed for the output projection.
- Every tile is written-once-read-later at whole-tile granularity (per-block
  / per-half tiles) to avoid false WAR/RAW serialization in Tile's tracker.
"""
import numpy as np
import ml_dtypes

B, N, M = 2, 4096, 4096
QUERY_DIM, CONTEXT_DIM = 512, 768
H, D = 8, 64
INNER = H * D  # 512
NCORES = 8
N_DEV = (B * N) // NCORES  # 1024 query rows per core
M_PAD_MIN = 1792

BF16 = ml_dtypes.bfloat16
SCALE = float(D) ** -0.5
FP8_PRE = 16.0  # fp8 prescale on Q and K; exp scale divides by 16*16

_compiled = {}
_DBG = False


def _build(m_pad):
    from concourse import bacc
    import concourse.bass as bass
    import concourse.mybir as mybir
    import concourse.tile as tile

    F32 = mybir.dt.float32
    BF = mybir.dt.bfloat16
    F8 = mybir.dt.float8e4
    AF = mybir.ActivationFunctionType
    PM = mybir.MatmulPerfMode

    KT = m_pad // 128  # k-tiles (17 for 2176)
    # context blocks of up to 4 k-tiles
    BLOCKS = [(4 * g, min(4, KT - 4 * g)) for g in range((KT + 3) // 4)]
    NB = len(BLOCKS)
    BA = 2  # pass A covers blocks [0, BA), pass B the rest
    CQ = QUERY_DIM // 128  # 4
    CC = CONTEXT_DIM // 128  # 6
    CI = INNER // 128  # 4
    EXP_SCALE = SCALE / (FP8_PRE * FP8_PRE)

    # exp groups of up to 3 k-tiles within each pass (psum: 3 banks x 2 bufs)
    def make_groups(kts):
        out = []
        i = 0
        while i < len(kts):
            out.append(kts[i : i + 3])
            i += 3
        return out

    A_KT = [kt for g0, gn in BLOCKS[:BA] for kt in range(g0, g0 + gn)]
    B_KT = [kt for g0, gn in BLOCKS[BA:] for kt in range(g0, g0 + gn)]
    GR_A = make_groups(A_KT)  # e.g. [[0,1,2],[3,4,5],[6,7]]
    GR_B = make_groups(B_KT)  # e.g. [[8,9,10],[11,12,13],[14,15,16]]

    nc = bacc.Bacc()
    xs_d = nc.declare_dram_parameter("xs", [N_DEV, QUERY_DIM], BF, isOutput=False)
    ctx_d = nc.declare_dram_parameter("ctx", [m_pad, CONTEXT_DIM], BF, isOutput=False)
    val_d = nc.declare_dram_parameter("valid", [m_pad], BF, isOutput=False)
    wqk_d = nc.declare_dram_parameter(
        "Wqk", [QUERY_DIM + CONTEXT_DIM, INNER], BF, isOutput=False
    )
    wvo_d = nc.declare_dram_parameter(
        "Wvo", [CONTEXT_DIM + INNER, INNER], BF, isOutput=False
    )
    bo_d = nc.declare_dram_parameter("bo", [QUERY_DIM], F32, isOutput=False)
    out_d = nc.declare_dram_parameter("out", [N_DEV, QUERY_DIM], F32, isOutput=True)

    with tile.TileContext(nc) as tc:
        with (
            tc.tile_pool(name="big", bufs=1) as big,
            tc.tile_pool(name="pt", bufs=4) as ptp,
            tc.tile_pool(name="fin", bufs=3) as finp,
            tc.tile_pool(name="otp", bufs=2) as otp,
            tc.tile_pool(name="outp", bufs=2) as outp,
            tc.tile_pool(name="ps_sc", bufs=2, space="PSUM") as ps_sc,
            tc.tile_pool(name="ps_u", bufs=1, space="PSUM") as ps_u,
            tc.tile_pool(name="ps_kv", bufs=1, space="PSUM") as ps_kv,
        ):
            # ---- static SBUF tiles (each written once, at one spot) ----
            wqk = big.tile([128, CQ + CC, INNER], BF, tag="wqk", name="wqk")
            wvo = big.tile([128, CC + CI, INNER], BF, tag="wvo", name="wvo")
            wq = wqk[:, 0:CQ, :]
            wk = wqk[:, CQ : CQ + CC, :]
            wv = wvo[:, 0:CC, :]
            wo = wvo[:, CC : CC + CI, :]
            bo_bc = big.tile([128, QUERY_DIM], F32, tag="bo", name="bo")
            valid = big.tile([128, KT], BF, tag="valid", name="valid")
            xTh = [
                big.tile([128, CQ, 512], BF, tag=f"xT{i}", name=f"xT{i}")
                for i in range(2)
            ]
            MA = min(1024, m_pad)  # pass-A context columns
            MB = m_pad - MA
            ctxT0 = big.tile([128, CC, 512], BF, tag="cT0", name="cT0")
            ctxT1 = big.tile([128, CC, 512], BF, tag="cT1", name="cT1")
            ctxTB = big.tile([128, CC, MB], BF, tag="cTB", name="cTB")
            qT8h = [
                big.tile([128, CI, 512], F8, tag=f"qT8_{i}", name=f"qT8_{i}")
                for i in range(2)
            ]
            qT8fh = [
                big.tile([32, H, 2, 512], F8, tag=f"qT8f_{i}", name=f"qT8f_{i}")
                for i in range(2)
            ]
            kT8b = [
                big.tile([128, CI, 512], F8, tag=f"kT8b{i}", name=f"kT8b{i}")
                for i in range(2)
            ]
            kT8B = big.tile([128, CI, MB], F8, tag="kT8B", name="kT8B")
            kT8fb = [
                big.tile([32, H, 2, 512], F8, tag=f"kfb{i}", name=f"kfb{i}")
                for i in range(2)
            ]
            kT8fB = big.tile([32, H, 2, MB], F8, tag="kfB", name="kfB")
            v2t = [
                big.tile([128, H, 65], BF, tag=f"v2_{t}", name=f"v2_{t}")
                for t in range(KT)
            ]
            ua = [
                big.tile([128, 4, 65], F32, tag=f"ua{r}", name=f"ua{r}")
                for r in range(16)
            ]
            onat = [
                big.tile([128, 4, INNER], BF, tag=f"on{qb}", name=f"on{qb}")
                for qb in range(2)
            ]

            # ---- loads + input transposes (DMA device is serialized: Q path
            # first, then ctx blocks in consumption order) ----
            # Few, large DMAs: chains between unlike DMAs then cost little.
            nc.sync.dma_start_transpose(xTh[0][:], xs_d[0:512, :])
            nc.sync.dma_start_transpose(xTh[1][:], xs_d[512:1024, :])
            nc.sync.dma_start_transpose(ctxT0[:], ctx_d[0:512, :])
            nc.scalar.dma_start(
                out=wqk[:], in_=wqk_d[:].rearrange("(o p) f -> p o f", p=128)
            )
            nc.scalar.dma_start(
                out=valid[:], in_=val_d[:].rearrange("(t p) -> p t", p=128)
            )
            nc.sync.dma_start_transpose(ctxT1[:], ctx_d[512:1024, :])

            # rotating psum slots for the prologue projection chains: the
            # two 3-bank sc tiles provide 6 independent banks
            _slot_state = {"tile": None, "i": 0}

            def next_slot():
                if _slot_state["i"] % 3 == 0:
                    _slot_state["tile"] = ps_sc.tile(
                        [128, 3, 512], F32, tag="sc", name="slot"
                    )
                j = _slot_state["i"] % 3
                _slot_state["i"] += 1
                return _slot_state["tile"][:, j, :]

            # ---- Q projection unit: one (dc, qf) chain -> fp8 drain ----
            def q_unit(dc, qf):
                pq = next_slot()
                for c in range(CQ):
                    nc.tensor.matmul(
                        pq,
                        wq[:, c, dc * 128 : (dc + 1) * 128],
                        xTh[qf][:, c, :],
                        start=(c == 0),
                        stop=(c == CQ - 1),
                    )
                nc.vector.tensor_scalar_mul(qT8h[qf][:, dc, :], pq, FP8_PRE)

            def q_fold(qf):
                # one DMA per 32-partition quarter: all 4 dc chunks at once
                # (dst heads h = 2*dc + j//2 are stride-2 in h)
                dst5 = qT8fh[qf][:].rearrange(
                    "p (hh hl) i m -> p hl hh i m", hl=2
                )
                for j in range(4):
                    nc.sync.dma_start(
                        out=dst5[:, j // 2, :, j % 2, :],
                        in_=qT8h[qf][32 * j : 32 * j + 32, :, :],
                    )

            # ---- K/V production units (blocks 0,1 separate; B merged) ----
            def ctx_of(base):
                if base < 512:
                    return ctxT0, base
                if base < MA:
                    return ctxT1, base - 512
                return ctxTB, base - MA

            def k_unit(g, dc, use_slot=False):
                g0, gn = BLOCKS[g]
                bw = gn * 128
                ctx_t, lo = ctx_of(g0 * 128)
                k_t, klo = (
                    (kT8b[g], 0) if g < 2 else (kT8B, g0 * 128 - MA)
                )
                pk = (
                    next_slot()
                    if use_slot
                    else ps_kv.tile([128, 512], F32, tag="kv", name="pk")[:]
                )
                for c in range(CC):
                    nc.tensor.matmul(
                        pk[:, 0:bw],
                        wk[:, c, dc * 128 : (dc + 1) * 128],
                        ctx_t[:, c, lo : lo + bw],
                        start=(c == 0),
                        stop=(c == CC - 1),
                    )
                nc.vector.tensor_scalar_mul(
                    k_t[:, dc, klo : klo + bw], pk[:, 0:bw], FP8_PRE
                )

            def k_fold(part):
                k_t, k_f = (
                    (kT8b[part], kT8fb[part]) if part < 2 else (kT8B, kT8fB)
                )
                dst5 = k_f[:].rearrange("p (hh hl) i m -> p hl hh i m", hl=2)
                for j in range(4):
                    nc.sync.dma_start(
                        out=dst5[:, j // 2, :, j % 2, :],
                        in_=k_t[32 * j : 32 * j + 32, :, :],
                    )

            def v_unit(kt):
                ctx_t, lo = ctx_of(kt * 128)
                pv = ps_kv.tile([128, 512], F32, tag="kv", name="pv")
                for c in range(CC):
                    nc.tensor.matmul(
                        pv[:],
                        ctx_t[:, c, lo : lo + 128],
                        wv[:, c, :],
                        start=(c == 0),
                        stop=(c == CC - 1),
                    )
                v2h = v2t[kt]
                nc.vector.tensor_copy(
                    v2h[:, :, 0:64], pv[:].rearrange("p (h d) -> p h d", d=64)
                )
                nc.vector.tensor_copy(
                    v2h[:, :, 64:65], valid[:, kt : kt + 1].to_broadcast([128, H, 1])
                )

            # PE warmup: dep-free matmuls ramp the p-state while the first
            # DMAs land; results go to a psum bank nobody reads
            # PE warmup: dep-free matmuls ramp the p-state while DMAs land
            dummy = big.tile([128, 512], BF, tag="dummy", name="dummy")
            nc.vector.memset(dummy[:], 0.0)
            wps = ps_kv.tile([128, 512], F32, tag="kv", name="wps")
            for i in range(24):
                nc.tensor.matmul(
                    wps[:], dummy[:, 0:128], dummy[:], start=True, stop=True
                )
            # Q half 0 first (gates rounds 0-7), then K blocks, then Q half 1
            for dc in range(CI):
                q_unit(dc, 0)
            q_fold(0)
            for dc in range(CI):
                k_unit(0, dc, use_slot=True)
            k_fold(0)
            for dc in range(CI):
                k_unit(1, dc, use_slot=True)
            k_fold(1)
            nc.gpsimd.dma_start(
                out=wvo[:], in_=wvo_d[:].rearrange("(o p) f -> p o f", p=128)
            )
            nc.gpsimd.dma_start(
                out=bo_bc[:],
                in_=bass.AP(tensor=bo_d, offset=0, ap=[[0, 128], [1, QUERY_DIM]]),
            )
            nc.sync.dma_start_transpose(ctxTB[:], ctx_d[MA:m_pad, :])

            # production consumed later (Q half 1 at r8; blocks 2+ in pass B),
            # emitted inside pass-A rounds r>=1
            deferred = []
            for dc in range(CI):
                deferred.append(lambda dc=dc: q_unit(dc, 1))
            deferred.append(lambda: q_fold(1))
            for g in range(BA, NB):
                for dc in range(CI):
                    deferred.append(lambda g=g, dc=dc: k_unit(g, dc))
                for kt in range(BLOCKS[g][0], BLOCKS[g][0] + BLOCKS[g][1]):
                    deferred.append(lambda kt=kt: v_unit(kt))
            deferred.append(lambda: k_fold(2))
            di = [0]

            def emit_units(k):
                while k > 0 and di[0] < len(deferred):
                    deferred[di[0]]()
                    di[0] += 1
                    k -= 1

            # ---- attention rounds ----
            def do_pass(qb, h, groups, upsum, interleave, weave=None,
                        dbg_cap=None, two_phase=False):
                last_g = len(groups) - 1

                def scores_exp(gi, kts):
                    gn = len(kts)
                    sc = ps_sc.tile([128, 3, 512], F32, tag="sc", name="sc")
                    for j, kt in enumerate(kts):
                        base = kt * 128
                        if base < 512:
                            k_f, lo = kT8fb[0], base
                        elif base < MA:
                            k_f, lo = kT8fb[1], base - 512
                        else:
                            k_f, lo = kT8fB, base - MA
                        nc.tensor.matmul(
                            sc[:, j, :],
                            k_f[:, h, :, lo : lo + 128],
                            qT8fh[qb][:, h, :, :],
                            start=True,
                            stop=True,
                            perf_mode=PM.DoubleRow,
                        )
                    pt = ptp.tile([128, 3, 512], BF, tag="pt", name="pt")
                    nc.scalar.activation(
                        pt[:, 0:gn, :], sc[:, 0:gn, :], AF.Exp, scale=EXP_SCALE
                    )
                    return pt

                def pv(gi, kts, pt):
                    gn = len(kts)
                    for j, kt in enumerate(kts):
                        for qt in range(4):
                            nc.tensor.matmul(
                                upsum[:, qt, 0:65],
                                pt[:, j, qt * 128 : (qt + 1) * 128],
                                v2t[kt][:, h, :],
                                start=False,
                                stop=(gi == last_g) and (j == gn - 1),
                                skip_group_check=True,
                            )

                if two_phase:
                    pts = [scores_exp(gi, kts) for gi, kts in enumerate(groups)]
                    for gi, kts in enumerate(groups):
                        if weave is not None:
                            with tc.high_priority(offset=-1000000):
                                for fn_ in weave.get(gi, ()):
                                    fn_()
                        pv(gi, kts, pts[gi])
                    return
                for gi, kts in enumerate(groups):
                    pt = scores_exp(gi, kts)
                    if weave is not None:
                        with tc.high_priority(offset=-1000000):
                            for fn_ in weave.get(gi, ()):
                                fn_()
                    if interleave:
                        with tc.high_priority(offset=-1000000):
                            emit_units(2 if di[0] < 13 else 1)
                    pv(gi, kts, pt)

            # pass A: U_a -> SBUF.  Round 0 weaves in K block 1 (+fold) and
            # the A-side V units just ahead of their first consumers.
            a_kts = GR_A
            weave0 = {
                gi: [lambda kt=kt: v_unit(kt) for kt in a_kts[gi]]
                for gi in range(len(a_kts))
            }

            dbg_pt = []
            for r in range(16):
                qb, h = r // 8, r % 8
                ua_ps = ps_u.tile([128, 4, 128], F32, tag="u", name="ua_ps")
                nc.vector.memset(ua_ps[:], 0.0)
                do_pass(
                    qb, h, GR_A, ua_ps, r >= 1,
                    weave=weave0 if r == 0 else None,
                    two_phase=(r == 0),
                )
                nc.vector.tensor_copy(ua[r][:], ua_ps[:, :, 0:65])

            # pass B: U_b + U_a -> normalize -> O; out-proj per query block
            def out_proj_qt(qb, qt):
                oT = otp.tile([128, CI, 128], BF, tag="oT", name="oT")
                nc.sync.dma_start_transpose(oT[:], onat[qb][:, qt, :])
                po = ps_kv.tile([128, 512], F32, tag="kv", name="po")
                for c in range(CI):
                    nc.tensor.matmul(
                        po[:],
                        oT[:, c, :],
                        wo[:, c, :],
                        start=(c == 0),
                        stop=(c == CI - 1),
                    )
                ot = outp.tile([128, QUERY_DIM], F32, tag="ot", name="ot")
                nc.vector.tensor_add(ot[:], po[:], bo_bc[:])
                qrow = (qb * 4 + qt) * 128
                nc.sync.dma_start(out=out_d[qrow : qrow + 128, :], in_=ot[:])

            for r in range(16):
                qb, h = r // 8, r % 8
                if r % 2 == 1:
                    ub_ps = ps_kv.tile([128, 4, 128], F32, tag="kv", name="ub_ps")
                else:
                    ub_ps = ps_u.tile([128, 4, 128], F32, tag="u", name="ub_ps")
                nc.vector.memset(ub_ps[:], 0.0)
                do_pass(qb, h, GR_B, ub_ps, False)
                fin = finp.tile([128, 4, 65], F32, tag="fin", name="fin")
                if r == 15:
                    # last round: per-qtile chains so the output projection
                    # starts as soon as each qtile's normalize lands
                    for qt in range(4):
                        nc.vector.tensor_add(
                            fin[:, qt, :], ub_ps[:, qt, 0:65], ua[r][:, qt, :]
                        )
                        nc.vector.reciprocal(
                            fin[:, qt, 64:65], fin[:, qt, 64:65]
                        )
                        nc.vector.tensor_scalar_mul(
                            onat[qb][:, qt, h * 64 : (h + 1) * 64],
                            fin[:, qt, 0:64],
                            fin[:, qt, 64:65],
                        )
                else:
                    nc.vector.tensor_add(fin[:], ub_ps[:, :, 0:65], ua[r][:])
                    nc.vector.reciprocal(fin[:, :, 64:65], fin[:, :, 64:65])
                    for qt in range(4):
                        nc.vector.tensor_scalar_mul(
                            onat[qb][:, qt, h * 64 : (h + 1) * 64],
                            fin[:, qt, 0:64],
                            fin[:, qt, 64:65],
                        )
                if r >= 8 and r % 2 == 0:
                    out_proj_qt(0, (r - 8) // 2)
            # final block: batch transposes, then matmul+add chains, then stores
            otf = big.tile([128, 4, QUERY_DIM], F32, tag="otf", name="otf")
            for qt in range(4):
                oT = otp.tile([128, CI, 128], BF, tag="oTf", name=f"oTf{qt}")
                nc.sync.dma_start_transpose(oT[:], onat[1][:, qt, :])
                po = ps_sc.tile([128, 3, 512], F32, tag="sc", name="pof")
                for c in range(CI):
                    nc.tensor.matmul(
                        po[:, 0, :],
                        oT[:, c, :],
                        wo[:, c, :],
                        start=(c == 0),
                        stop=(c == CI - 1),
                    )
                nc.vector.tensor_add(otf[:, qt, :], po[:, 0, :], bo_bc[:])
            nc.sync.dma_start(
                out=out_d[512:1024, :].rearrange("(t p) f -> p t f", p=128),
                in_=otf[:],
            )

            if _DBG:
                dumps = [
                    ("d_qT8f0", qT8fh[0], [32, H, 2, 512]),
                    ("d_kT8fb0", kT8fb[0], [32, H, 2, 512]),
                    ("d_kT8b0", kT8b[0], [128, CI, 512]),
                    ("d_v2_0", v2t[0], [128, H, 65]),
                    ("d_ua0", ua[0], [128, 4, 65]),
                    ("d_on0", onat[0], [128, 4, INNER]),
                    ("d_xT0", xTh[0], [128, CQ, 512]),
                    ("d_cT0", ctxT0, [128, CC, 512]),
                ]
                for nm, t, shp in dumps:
                    dt_ = t[:].dtype
                    dd = nc.declare_dram_parameter(nm, shp, dt_, isOutput=True)
                    nc.sync.dma_start(out=dd[:], in_=t[:])

    nc.compile()
    return nc


def kernel(x, context_tensor, mask, Wq, Wk, Wv, Wo, bo):
    from concourse.bass_utils import run_bass_kernel_spmd

    x = np.asarray(x, dtype=np.float32)
    context_tensor = np.asarray(context_tensor, dtype=np.float32)
    mask = np.asarray(mask)
    Wq = np.asarray(Wq, dtype=np.float32)
    Wk = np.asarray(Wk, dtype=np.float32)
    Wv = np.asarray(Wv, dtype=np.float32)
    Wo = np.asarray(Wo, dtype=np.float32)
    bo = np.ascontiguousarray(np.asarray(bo, dtype=np.float32))

    # host-side context compaction using the mask
    meffs = [int(mask[b].sum()) for b in range(B)]
    m_pad = max(M_PAD_MIN, ((max(meffs) + 127) // 128) * 128)
    ctx_c = np.zeros((B, m_pad, CONTEXT_DIM), dtype=BF16)
    val = np.zeros((B, m_pad), dtype=BF16)
    for b in range(B):
        idx = np.flatnonzero(mask[b])
        ctx_c[b, : len(idx)] = context_tensor[b, idx].astype(BF16)
        val[b, : len(idx)] = 1.0

    if m_pad not in _compiled:
        _compiled[m_pad] = _build(m_pad)
    nc = _compiled[m_pad]

    x16 = np.ascontiguousarray(x.astype(BF16))
    wqk16 = np.ascontiguousarray(np.concatenate([Wq, Wk], axis=0).astype(BF16))
    wvo16 = np.ascontiguousarray(np.concatenate([Wv, Wo], axis=0).astype(BF16))

    rows_per_core = N // (NCORES // B)  # 1024
    in_maps = []
    for d in range(NCORES):
        b = d // (NCORES // B)
        r0 = (d % (NCORES // B)) * rows_per_core
        in_maps.append(
            {
                "xs": np.ascontiguousarray(x16[b, r0 : r0 + rows_per_core]),
                "ctx": ctx_c[b],
                "valid": val[b],
                "Wqk": wqk16,
                "Wvo": wvo16,
                "bo": bo,
            }
        )

    res = run_bass_kernel_spmd(nc, in_maps, list(range(NCORES)))
    global _last_results
    _last_results = res
    out = np.empty((B, N, QUERY_DIM), dtype=np.float32)
    for d in range(NCORES):
        b = d // (NCORES // B)
        r0 = (d % (NCORES // B)) * rows_per_core
        out[b, r0 : r0 + rows_per_core] = res.results[d]["out"]
    return out
